# revision 26
# baseline (speedup 1.0000x reference)
"""Trainium2 Bass kernel for the Hebbian scatter-memory module.

Strategy: pure data-parallel over batch (1024 items -> 8 cores x 128 items).
Per core, items are processed in groups of 4 (4 x 32 tokens = 128 partitions).

Math per item b (reference):
  k    = key[b] @ Wk + bk                      [32,128]
  v    = (value[b] @ Wv + bv) * mod[b]         [32,128]
  corr = k^T v ; reg = k^T k
  dw   = A*(1-w)*corr - B*(reg @ w)
  w'   = w + m[b]*dw
  q    = (query[b] @ Wq + bq) -> [4,128]
  out  = (q @ w').flat @ Wagg + bagg

Device mapping (all matmuls bf16 inputs, fp32 PSUM accumulate):
  - host folds done_mask into key (m^2 == m for binary masks), transposes
    key/value to feature-major [K, tok] so no on-chip transposes are needed
  - reg @ w is computed associatively as k^T (k @ w) via the feature-major
    encoder output kT, avoiding a [128,512] PSUM->SBUF copy of reg
  - elementwise chain (5 ops): X = corr*A ; negZ = (w-1)*X ;
    U = rw*B ; Vp = negZ + U (= -dw) ; w' = w - Vp
  - read: w'^T q = w^T q + Vp^T (-q), two accumulated matmuls per item,
    so no bf16 cast of w' is needed
"""

import os
from contextlib import ExitStack

import numpy as np

import concourse.bass as bass
import concourse.bacc as bacc
import concourse.mybir as mybir
import concourse.tile as tile
from concourse.bass_utils import run_bass_kernel_spmd

NCORES = 8
B = 1024
NTOK = 32
K = 128
V = 128
H = 4
BL = B // NCORES          # items per core
GI = 4                    # items per group (4*32 tokens = 128)
NG = BL // GI             # groups per core
CH = 32                   # items per read-aggregation chunk
NCH = BL // CH            # chunks per core
GPC = CH // GI            # groups per chunk

F32 = mybir.dt.float32
BF16 = mybir.dt.bfloat16
NPBF = mybir.dt.np(BF16)

AOP = mybir.AluOpType
AF = mybir.ActivationFunctionType

_prog_cache = {}
_last_bkr = None  # BassKernelResults of the most recent run (for test harness)


def _ensure_axon_hooks():
    """Provide antenv.axon_hooks if the image lacks it (needed only when
    BASS_TRACE profiling is requested; inert otherwise)."""
    try:
        import antenv.axon_hooks  # noqa: F401
        return
    except ImportError:
        pass
    import types
    import ctypes
    import contextlib
    import sys

    mod = types.ModuleType("antenv.axon_hooks")
    holder = {"h": None}
    mod.set_axon_ntff_profile_hook = lambda h: holder.__setitem__("h", h)
    mod.get_axon_ntff_profile_hook = lambda: holder["h"]

    so = "/opt/axon/libaxon_pjrt.so"
    if os.path.exists(so):
        try:
            lib = ctypes.CDLL(so)
            if hasattr(lib, "axon_start_nrt_profile"):
                lib.axon_start_nrt_profile.argtypes = [
                    ctypes.POINTER(ctypes.c_int64), ctypes.c_size_t]
                lib.axon_start_nrt_profile.restype = ctypes.c_int64
                lib.axon_stop_nrt_profile.argtypes = [ctypes.c_char_p]
                lib.axon_stop_nrt_profile.restype = ctypes.c_int64

                @contextlib.contextmanager
                def _hook(output_dir, device_ids):
                    import jax
                    jax.devices()
                    if device_ids:
                        ids = (ctypes.c_int64 * len(device_ids))(*device_ids)
                        rc = lib.axon_start_nrt_profile(ids, len(device_ids))
                    else:
                        rc = lib.axon_start_nrt_profile(None, 0)
                    if rc != 0:
                        raise RuntimeError(f"axon_start_nrt_profile rc={rc}")
                    try:
                        yield
                    finally:
                        n = lib.axon_stop_nrt_profile(str(output_dir).encode())
                        print(f"profile: {n} file(s) written to {output_dir}")

                holder["h"] = _hook
        except Exception:
            pass

    import antenv
    antenv.axon_hooks = mod
    sys.modules["antenv.axon_hooks"] = mod


def _build_program(with_bias, na):
    nc = bacc.Bacc()

    keyT_d = nc.dram_tensor("keyT", [NG, K, GI * NTOK], BF16, kind="ExternalInput")
    valT_d = nc.dram_tensor("valT", [NG, V, GI * NTOK], BF16, kind="ExternalInput")
    w_d = nc.dram_tensor("w_in", [K, BL * V], F32, kind="ExternalInput")
    wbf_d = nc.dram_tensor("w_in_bf", [K, BL * V], BF16, kind="ExternalInput")
    mod_d = nc.dram_tensor("modv", [GI * NTOK, NG], F32, kind="ExternalInput")
    qraw_d = nc.dram_tensor("qraw", [K, BL], BF16, kind="ExternalInput")
    wk_d = nc.dram_tensor("wk", [K, K], BF16, kind="ExternalInput")
    wv_d = nc.dram_tensor("wv", [V, V], BF16, kind="ExternalInput")
    wq_d = nc.dram_tensor("wq", [K, H * K], BF16, kind="ExternalInput")
    wagg_d = nc.dram_tensor("wagg", [V, H * V], BF16, kind="ExternalInput")
    arep_d = nc.dram_tensor("arep", [K, GI * V], BF16, kind="ExternalInput")
    brep_d = nc.dram_tensor("brep", [K, GI * V], BF16, kind="ExternalInput")
    id_d = nc.dram_tensor("ident", [128, 128], F32, kind="ExternalInput")
    if with_bias:
        ones_d = nc.dram_tensor("ones", [1, 512], BF16, kind="ExternalInput")
        bk_d = nc.dram_tensor("bk", [1, K], BF16, kind="ExternalInput")
        bv_d = nc.dram_tensor("bv", [1, V], BF16, kind="ExternalInput")
        bq_d = nc.dram_tensor("bq", [1, H * K], BF16, kind="ExternalInput")
        bagg_d = nc.dram_tensor("bagg", [1, V], BF16, kind="ExternalInput")
        mrow_d = nc.dram_tensor("mrow", [1, NG * GI * NTOK], BF16, kind="ExternalInput")

    wout_d = nc.dram_tensor("w_out", [K, BL * V], F32, kind="ExternalOutput")
    out_d = nc.dram_tensor("outp", [BL, V], F32, kind="ExternalOutput")

    with TileKernel(nc) as tc, ExitStack() as ctx:
        const = ctx.enter_context(tc.tile_pool(name="const", bufs=1))

        def cload(shape, dtype, src, tag):
            t = const.tile(shape, dtype, tag=tag)
            nc.sync.dma_start(t[:], src)
            return t

        wk_sb = cload([K, K], BF16, wk_d[:, :], "c_wk")
        wv_sb = cload([V, V], BF16, wv_d[:, :], "c_wv")
        wq_sb = cload([K, H * K], BF16, wq_d[:, :], "c_wq")
        # [v, (h, o)] layout so wagg_sb[:, 128h:+128] is lhsT for head h
        wagg_sb = cload([V, H * V], BF16, wagg_d[:, :], "c_wagg")
        arep_sb = cload([K, GI * V], BF16, arep_d[:, :], "c_arep")
        brep_sb = cload([K, GI * V], BF16, brep_d[:, :], "c_brep")
        id_sb = cload([128, 128], F32, id_d[:, :], "c_id")
        mod_sb = cload([GI * NTOK, NG], F32, mod_d[:, :], "c_mod")
        qraw_sb = cload([K, BL], BF16, qraw_d[:, :], "c_qraw")
        if with_bias:
            ones_sb = cload([1, 512], BF16, ones_d[:, :], "c_ones")
            bk_sb = cload([1, K], BF16, bk_d[:, :], "c_bk")
            bv_sb = cload([1, V], BF16, bv_d[:, :], "c_bv")
            bq_sb = cload([1, H * K], BF16, bq_d[:, :], "c_bq")
            bagg_sb = cload([1, V], BF16, bagg_d[:, :], "c_bagg")
            mrow_sb = cload([1, NG * GI * NTOK], BF16, mrow_d[:, :], "c_mrow")

        inp = ctx.enter_context(tc.tile_pool(name="inp", bufs=4))
        work = ctx.enter_context(tc.tile_pool(name="work", bufs=4))
        rds = ctx.enter_context(tc.tile_pool(name="rds", bufs=2))
        encp = ctx.enter_context(tc.tile_pool(name="encp", bufs=2, space="PSUM"))
        cwp = ctx.enter_context(tc.tile_pool(name="cwp", bufs=1, space="PSUM"))
        rdp = ctx.enter_context(tc.tile_pool(name="rdp", bufs=2, space="PSUM"))

        # ---- query encoder: qT_sb[k2, 4*item+h] = (query @ Wq + bq)^T ----
        qT_sb = const.tile([K, H * BL], BF16, tag="c_qT")
        qTn_sb = const.tile([K, H * BL], BF16, tag="c_qTn")
        if True:
            qenc = encp.tile([128, 512], F32, tag="enc")
            for h in range(H):
                nc.tensor.matmul(qenc[:, 128 * h:128 * (h + 1)],
                                 lhsT=wq_sb[:, 128 * h:128 * (h + 1)],
                                 rhs=qraw_sb[:], start=True, stop=not with_bias)
                if with_bias:
                    nc.tensor.matmul(qenc[:, 128 * h:128 * (h + 1)],
                                     lhsT=bq_sb[:, 128 * h:128 * (h + 1)],
                                     rhs=ones_sb[:, :BL], start=False, stop=True)
            qTr = qT_sb[:].rearrange("k (i h) -> k h i", h=H)
            for h in range(H):
                nc.vector.tensor_copy(qTr[:, h, :], qenc[:, 128 * h:128 * (h + 1)])
        nc.vector.tensor_scalar_mul(qTn_sb[:], qT_sb[:], -1.0)

        def load_w(gg):
            wbf = inp.tile([K, GI * V], BF16, tag="wbf", name=f"wbf_{gg}")
            nc.sync.dma_start(wbf[:], wbf_d[:, GI * V * gg:GI * V * (gg + 1)])
            if gg >= na:
                return None, wbf
            w_t = inp.tile([K, GI * V], F32, tag="w", name=f"w_t_{gg}")
            nc.sync.dma_start(w_t[:], w_d[:, GI * V * gg:GI * V * (gg + 1)])
            return w_t, wbf

        w_cur = load_w(0)
        for c in range(NCH):
            readT = rdp.tile([128, H * CH], F32, tag="rd")
            for g in range(GPC):
                gg = c * GPC + g
                w_nxt = load_w(gg + 1) if gg + 1 < NG else None
                w_t, wbf = w_cur
                if gg >= na:
                    # passive group: done_mask == 0 -> w_new = w; only the
                    # read path is needed, and w_new is filled host-side
                    for i in range(GI):
                        it = gg * GI + i
                        co = H * (g * GI + i)
                        nc.tensor.matmul(readT[:, co:co + H],
                                         lhsT=wbf[:, 128 * i:128 * (i + 1)],
                                         rhs=qT_sb[:, H * it:H * (it + 1)],
                                         start=True, stop=True)
                    w_cur = w_nxt
                    continue
                keyT_t = inp.tile([K, GI * NTOK], BF16, tag="keyT")
                nc.sync.dma_start(keyT_t[:], keyT_d[gg])
                valT_t = inp.tile([V, GI * NTOK], BF16, tag="valT")
                nc.sync.dma_start(valT_t[:], valT_d[gg])

                enc = encp.tile([128, 512], F32, tag="enc")
                # kenc [tok, i]
                nc.tensor.matmul(enc[:, 0:128], lhsT=keyT_t[:], rhs=wk_sb[:],
                                 start=True, stop=not with_bias)
                # kT [i, tok]
                nc.tensor.matmul(enc[:, 128:256], lhsT=wk_sb[:], rhs=keyT_t[:],
                                 start=True, stop=not with_bias)
                # venc [tok, v]
                nc.tensor.matmul(enc[:, 256:384], lhsT=valT_t[:], rhs=wv_sb[:],
                                 start=True, stop=not with_bias)
                if with_bias:
                    # masked bias: k = key_m @ Wk + m*bk  (mask folded into key on host)
                    mrow_g = mrow_sb[:, 128 * gg:128 * (gg + 1)]
                    nc.tensor.matmul(enc[:, 0:128], lhsT=mrow_g, rhs=bk_sb[:, :],
                                     start=False, stop=True)
                    nc.tensor.matmul(enc[:, 128:256], lhsT=bk_sb[:, :], rhs=mrow_g,
                                     start=False, stop=True)
                    nc.tensor.matmul(enc[:, 256:384], lhsT=ones_sb[:, :GI * NTOK],
                                     rhs=bv_sb[:, :], start=False, stop=True)

                k_sb = work.tile([GI * NTOK, K], BF16, tag="k")
                nc.scalar.copy(k_sb[:], enc[:, 0:128])
                kT_sb = work.tile([K, GI * NTOK], BF16, tag="kT")
                nc.scalar.copy(kT_sb[:], enc[:, 128:256])
                v_sb = work.tile([GI * NTOK, V], BF16, tag="v")
                nc.scalar.activation(v_sb[:], enc[:, 256:384], AF.Copy,
                                     scale=mod_sb[:, gg:gg + 1])

                # G = k @ w per item, col-tiled into partitions 32i..32i+31
                for i in range(GI):
                    nc.tensor.matmul(enc[32 * i:32 * (i + 1), 384:512],
                                     lhsT=kT_sb[:, 32 * i:32 * (i + 1)],
                                     rhs=wbf[:, 128 * i:128 * (i + 1)],
                                     start=True, stop=True, tile_position=(0, 32 * i))
                G_sb = work.tile([GI * NTOK, V], BF16, tag="G_sb")
                nc.scalar.copy(G_sb[:], enc[:, 384:512])

                # corr/rw: per-item row-tiled matmuls, one PSUM bank per item
                # (concurrent row-group matmuls must not share a bank on HW);
                # item i's bank holds corr at cols 0:128, rw at cols 128:256
                cw = cwp.tile([128, GI * 512], F32, tag="cw")
                for i in range(GI):
                    sl = slice(32 * i, 32 * (i + 1))
                    nc.tensor.matmul(cw[:, 512 * i:512 * i + 128],
                                     lhsT=k_sb[sl, :], rhs=v_sb[sl, :],
                                     start=True, stop=True, tile_position=(32 * i, 0))
                    nc.tensor.matmul(cw[:, 512 * i + 128:512 * i + 256],
                                     lhsT=k_sb[sl, :], rhs=G_sb[sl, :],
                                     start=True, stop=True, tile_position=(32 * i, 0))
                cw3 = cw[:].rearrange("p (g c) -> p g c", g=GI)

                X = work.tile([K, GI * V], BF16, tag="X")
                X3 = X[:].rearrange("p (g c) -> p g c", g=GI)
                A3 = arep_sb[:].rearrange("p (g c) -> p g c", g=GI)
                B3 = brep_sb[:].rearrange("p (g c) -> p g c", g=GI)
                nc.vector.tensor_tensor(X3, cw3[:, :, 0:128], A3, op=AOP.mult)
                U = work.tile([K, GI * V], BF16, tag="U")
                U3 = U[:].rearrange("p (g c) -> p g c", g=GI)
                nc.vector.tensor_tensor(U3, cw3[:, :, 128:256], B3, op=AOP.mult)
                wm1 = work.tile([K, GI * V], BF16, tag="wm1")
                nc.vector.tensor_scalar_sub(wm1[:], wbf[:], 1.0)
                negZ = work.tile([K, GI * V], BF16, tag="negZ")
                nc.vector.tensor_tensor(negZ[:], wm1[:], X[:], op=AOP.mult)
                Vp = work.tile([K, GI * V], BF16, tag="Vp")
                nc.gpsimd.tensor_tensor(Vp[:], negZ[:], U[:], op=AOP.add)
                wnew = work.tile([K, GI * V], F32, tag="wnew")
                nc.gpsimd.tensor_tensor(wnew[:], w_t[:], Vp[:], op=AOP.subtract)
                nc.sync.dma_start(wout_d[:, GI * V * gg:GI * V * (gg + 1)],
                                  wnew[:])

                # readT[v, 4*il+h] = (w' ^T q)[v,h] = w^T q - Vp^T q
                for i in range(GI):
                    it = gg * GI + i           # item within core
                    co = H * (g * GI + i)      # column offset within chunk
                    nc.tensor.matmul(readT[:, co:co + H],
                                     lhsT=wbf[:, 128 * i:128 * (i + 1)],
                                     rhs=qT_sb[:, H * it:H * (it + 1)],
                                     start=True, stop=False)
                    nc.tensor.matmul(readT[:, co:co + H],
                                     lhsT=Vp[:, 128 * i:128 * (i + 1)],
                                     rhs=qTn_sb[:, H * it:H * (it + 1)],
                                     start=False, stop=True)
                w_cur = w_nxt

            # ---- read aggregation for this chunk of 32 items ----
            readT_sb = rds.tile([128, H * CH], BF16, tag="rdsb")
            nc.vector.tensor_copy(readT_sb[:], readT[:])
            rr = readT_sb[:].rearrange("v (i h) -> v h i", h=H)
            aggT = rdp.tile([128, CH], F32, tag="rd")
            for h in range(H):
                nc.tensor.matmul(aggT[:, :], lhsT=wagg_sb[:, 128 * h:128 * (h + 1)],
                                 rhs=rr[:, h, :], start=(h == 0),
                                 stop=(h == H - 1) and not with_bias)
            if with_bias:
                nc.tensor.matmul(aggT[:, :], lhsT=bagg_sb[:, :],
                                 rhs=ones_sb[:, :CH], start=False, stop=True)
            aggT_sb = rds.tile([128, CH], F32, tag="aggsb")
            nc.vector.tensor_copy(aggT_sb[:], aggT[:])
            outT = rdp.tile([CH, 128], F32, tag="rd")
            nc.tensor.transpose(outT[:], aggT_sb[:], id_sb[:])
            out_sb = rds.tile([CH, 128], F32, tag="outsb")
            nc.scalar.copy(out_sb[:], outT[:])
            nc.sync.dma_start(out_d[CH * c:CH * (c + 1), :], out_sb[:])

    nc.finalize()
    return nc


def TileKernel(nc):
    return tile.TileContext(nc)


def _prep_inputs(value, key, modulation, query, w_assoc, done_mask,
                 Wk, bk, Wv, bv, A, B_mat, Wq, bq, Wagg, bagg, with_bias,
                 perms, na):
    mask = done_mask.astype(np.float32)
    key_m = (key * mask[:, None, None]).astype(np.float32)

    shared = {
        "wk": Wk.astype(NPBF),
        "wv": Wv.astype(NPBF),
        "wq": Wq.astype(NPBF),
        "wagg": np.ascontiguousarray(
            Wagg.reshape(H, K, V).transpose(1, 0, 2).reshape(V, H * V)).astype(NPBF),
        "arep": np.ascontiguousarray(np.tile(A, (1, GI))).astype(NPBF),
        "brep": np.ascontiguousarray(np.tile(B_mat, (1, GI))).astype(NPBF),
        "ident": np.eye(128, dtype=np.float32),
    }
    if with_bias:
        shared.update({
            "ones": np.ones((1, 512), NPBF),
            "bk": bk.reshape(1, K).astype(NPBF),
            "bv": bv.reshape(1, V).astype(NPBF),
            "bq": bq.reshape(1, H * K).astype(NPBF),
            "bagg": bagg.reshape(1, V).astype(NPBF),
        })

    in_maps = []
    nact = GI * na
    for c in range(NCORES):
        idx = c * BL + perms[c]
        aidx = idx[:nact]
        wperm = np.ascontiguousarray(
            w_assoc[idx].transpose(1, 0, 2).reshape(K, BL * V)).astype(np.float32)
        km = key_m[aidx].reshape(na, GI, NTOK, K)
        keyT = np.zeros((NG, K, GI * NTOK), np.float32)
        keyT[:na] = km.transpose(0, 3, 1, 2).reshape(na, K, GI * NTOK)
        valT = np.zeros((NG, V, GI * NTOK), np.float32)
        valT[:na] = value[aidx].reshape(na, GI, NTOK, V).transpose(0, 3, 1, 2).reshape(
            na, V, GI * NTOK)
        modv = np.zeros((GI * NTOK, NG), np.float32)
        modv[:, :na] = modulation[aidx, :, 0].reshape(na, GI * NTOK).T
        m = {
            "keyT": keyT.astype(NPBF),
            "valT": valT.astype(NPBF),
            "w_in": wperm,
            "w_in_bf": wperm.astype(NPBF),
            "modv": modv,
            "qraw": np.ascontiguousarray(query[idx].T).astype(NPBF),
            **shared,
        }
        if with_bias:
            mrow = mask[idx].reshape(NG * GI, 1).repeat(NTOK, axis=1)
            m["mrow"] = mrow.reshape(1, NG * GI * NTOK).astype(NPBF)
        in_maps.append(m)
    return in_maps


def kernel(value, key, modulation, query, w_assoc, done_mask,
           Wk, bk, Wv, bv, A, B_mat, Wq, bq, Wagg, bagg):
    global _last_bkr
    value = np.asarray(value, np.float32)
    key = np.asarray(key, np.float32)
    modulation = np.asarray(modulation, np.float32)
    query = np.asarray(query, np.float32)
    w_assoc = np.asarray(w_assoc, np.float32)
    done_mask_np = np.asarray(done_mask)

    with_bias = bool(np.any(bk) or np.any(bv) or np.any(bq) or np.any(bagg))

    maskf = (done_mask_np != 0)
    perms = []
    max_active = 0
    for c in range(NCORES):
        mc = maskf[c * BL:(c + 1) * BL]
        perm = np.argsort(~mc, kind="stable")
        perms.append(perm)
        max_active = max(max_active, int(mc.sum()))
    na = min(NG, (max_active + GI - 1) // GI)

    key_prog = (with_bias, na)
    if key_prog not in _prog_cache:
        _prog_cache[key_prog] = _build_program(with_bias, na)
    nc = _prog_cache[key_prog]

    in_maps = _prep_inputs(value, key, modulation, query, w_assoc, done_mask_np,
                           np.asarray(Wk), np.asarray(bk), np.asarray(Wv),
                           np.asarray(bv), np.asarray(A), np.asarray(B_mat),
                           np.asarray(Wq), np.asarray(bq), np.asarray(Wagg),
                           np.asarray(bagg), with_bias, perms, na)

    _ensure_axon_hooks()
    try:
        bkr = run_bass_kernel_spmd(nc, in_maps, list(range(NCORES)))
    except ModuleNotFoundError:
        os.environ["BASS_NEVER_TRACE"] = "1"
        bkr = run_bass_kernel_spmd(nc, in_maps, list(range(NCORES)))
    _last_bkr = bkr

    w_new = np.empty((B, K, V), np.float32)
    out = np.empty((B, V), np.float32)
    nact = GI * na
    for c in range(NCORES):
        idx = c * BL + perms[c]
        wc = bkr.results[c]["w_out"].reshape(K, BL, V).transpose(1, 0, 2)
        w_new[idx[:nact]] = wc[:nact]
        w_new[idx[nact:]] = w_assoc[idx[nact:]]
        out[idx] = bkr.results[c]["outp"]
    return out, w_new


# revision 27
# speedup vs baseline: 1.0559x; 1.0559x over previous
"""Trainium2 Bass kernel for the Hebbian scatter-memory module.

Strategy: pure data-parallel over batch (1024 items -> 8 cores x 128 items).
Per core, items are processed in groups of 4 (4 x 32 tokens = 128 partitions).

Math per item b (reference):
  k    = key[b] @ Wk + bk                      [32,128]
  v    = (value[b] @ Wv + bv) * mod[b]         [32,128]
  corr = k^T v ; reg = k^T k
  dw   = A*(1-w)*corr - B*(reg @ w)
  w'   = w + m[b]*dw
  q    = (query[b] @ Wq + bq) -> [4,128]
  out  = (q @ w').flat @ Wagg + bagg

Device mapping (all matmuls bf16 inputs, fp32 PSUM accumulate):
  - host folds done_mask into key (m^2 == m for binary masks), transposes
    key/value to feature-major [K, tok] so no on-chip transposes are needed
  - reg @ w is computed associatively as k^T (k @ w) via the feature-major
    encoder output kT, avoiding a [128,512] PSUM->SBUF copy of reg
  - elementwise chain (5 ops): X = corr*A ; negZ = (w-1)*X ;
    U = rw*B ; Vp = negZ + U (= -dw) ; w' = w - Vp
  - read: w'^T q = w^T q + Vp^T (-q), two accumulated matmuls per item,
    so no bf16 cast of w' is needed
"""

import os
from contextlib import ExitStack

import numpy as np

import concourse.bass as bass
import concourse.bacc as bacc
import concourse.mybir as mybir
import concourse.tile as tile
from concourse.bass_utils import run_bass_kernel_spmd

NCORES = 8
B = 1024
NTOK = 32
K = 128
V = 128
H = 4
BL = B // NCORES          # items per core
GI = 4                    # items per group (4*32 tokens = 128)
NG = BL // GI             # groups per core
CH = 32                   # items per read-aggregation chunk
NCH = BL // CH            # chunks per core
GPC = CH // GI            # groups per chunk

F32 = mybir.dt.float32
BF16 = mybir.dt.bfloat16
NPBF = mybir.dt.np(BF16)

AOP = mybir.AluOpType
AF = mybir.ActivationFunctionType

_prog_cache = {}
_last_bkr = None  # BassKernelResults of the most recent run (for test harness)


def _ensure_axon_hooks():
    """Provide antenv.axon_hooks if the image lacks it (needed only when
    BASS_TRACE profiling is requested; inert otherwise)."""
    try:
        import antenv.axon_hooks  # noqa: F401
        return
    except ImportError:
        pass
    import types
    import ctypes
    import contextlib
    import sys

    mod = types.ModuleType("antenv.axon_hooks")
    holder = {"h": None}
    mod.set_axon_ntff_profile_hook = lambda h: holder.__setitem__("h", h)
    mod.get_axon_ntff_profile_hook = lambda: holder["h"]

    so = "/opt/axon/libaxon_pjrt.so"
    if os.path.exists(so):
        try:
            lib = ctypes.CDLL(so)
            if hasattr(lib, "axon_start_nrt_profile"):
                lib.axon_start_nrt_profile.argtypes = [
                    ctypes.POINTER(ctypes.c_int64), ctypes.c_size_t]
                lib.axon_start_nrt_profile.restype = ctypes.c_int64
                lib.axon_stop_nrt_profile.argtypes = [ctypes.c_char_p]
                lib.axon_stop_nrt_profile.restype = ctypes.c_int64

                @contextlib.contextmanager
                def _hook(output_dir, device_ids):
                    import jax
                    jax.devices()
                    if device_ids:
                        ids = (ctypes.c_int64 * len(device_ids))(*device_ids)
                        rc = lib.axon_start_nrt_profile(ids, len(device_ids))
                    else:
                        rc = lib.axon_start_nrt_profile(None, 0)
                    if rc != 0:
                        raise RuntimeError(f"axon_start_nrt_profile rc={rc}")
                    try:
                        yield
                    finally:
                        n = lib.axon_stop_nrt_profile(str(output_dir).encode())
                        print(f"profile: {n} file(s) written to {output_dir}")

                holder["h"] = _hook
        except Exception:
            pass

    import antenv
    antenv.axon_hooks = mod
    sys.modules["antenv.axon_hooks"] = mod


def _build_program(with_bias, na):
    nc = bacc.Bacc()

    keyT_d = nc.dram_tensor("keyT", [NG, K, GI * NTOK], BF16, kind="ExternalInput")
    valT_d = nc.dram_tensor("valT", [NG, V, GI * NTOK], BF16, kind="ExternalInput")
    w_d = nc.dram_tensor("w_in", [K, BL * V], F32, kind="ExternalInput")
    wbf_d = nc.dram_tensor("w_in_bf", [K, BL * V], BF16, kind="ExternalInput")
    mod_d = nc.dram_tensor("modv", [GI * NTOK, NG], F32, kind="ExternalInput")
    qraw_d = nc.dram_tensor("qraw", [K, BL], BF16, kind="ExternalInput")
    wk_d = nc.dram_tensor("wk", [K, K], BF16, kind="ExternalInput")
    wv_d = nc.dram_tensor("wv", [V, V], BF16, kind="ExternalInput")
    wq_d = nc.dram_tensor("wq", [K, H * K], BF16, kind="ExternalInput")
    wagg_d = nc.dram_tensor("wagg", [V, H * V], BF16, kind="ExternalInput")
    arep_d = nc.dram_tensor("arep", [K, GI * V], BF16, kind="ExternalInput")
    brep_d = nc.dram_tensor("brep", [K, GI * V], BF16, kind="ExternalInput")
    id_d = nc.dram_tensor("ident", [128, 128], F32, kind="ExternalInput")
    if with_bias:
        ones_d = nc.dram_tensor("ones", [1, 512], BF16, kind="ExternalInput")
        bk_d = nc.dram_tensor("bk", [1, K], BF16, kind="ExternalInput")
        bv_d = nc.dram_tensor("bv", [1, V], BF16, kind="ExternalInput")
        bq_d = nc.dram_tensor("bq", [1, H * K], BF16, kind="ExternalInput")
        bagg_d = nc.dram_tensor("bagg", [1, V], BF16, kind="ExternalInput")
        mrow_d = nc.dram_tensor("mrow", [1, NG * GI * NTOK], BF16, kind="ExternalInput")

    wout_d = nc.dram_tensor("w_out", [K, BL * V], F32, kind="ExternalOutput")
    out_d = nc.dram_tensor("outp", [BL, V], F32, kind="ExternalOutput")

    with TileKernel(nc) as tc, ExitStack() as ctx:
        const = ctx.enter_context(tc.tile_pool(name="const", bufs=1))

        def cload(shape, dtype, src, tag):
            t = const.tile(shape, dtype, tag=tag)
            nc.sync.dma_start(t[:], src)
            return t

        wk_sb = cload([K, K], BF16, wk_d[:, :], "c_wk")
        wv_sb = cload([V, V], BF16, wv_d[:, :], "c_wv")
        wq_sb = cload([K, H * K], BF16, wq_d[:, :], "c_wq")
        # [v, (h, o)] layout so wagg_sb[:, 128h:+128] is lhsT for head h
        wagg_sb = cload([V, H * V], BF16, wagg_d[:, :], "c_wagg")
        arep_sb = cload([K, GI * V], BF16, arep_d[:, :], "c_arep")
        brep_sb = cload([K, GI * V], BF16, brep_d[:, :], "c_brep")
        id_sb = cload([128, 128], F32, id_d[:, :], "c_id")
        mod_sb = cload([GI * NTOK, NG], F32, mod_d[:, :], "c_mod")
        qraw_sb = cload([K, BL], BF16, qraw_d[:, :], "c_qraw")
        if with_bias:
            ones_sb = cload([1, 512], BF16, ones_d[:, :], "c_ones")
            bk_sb = cload([1, K], BF16, bk_d[:, :], "c_bk")
            bv_sb = cload([1, V], BF16, bv_d[:, :], "c_bv")
            bq_sb = cload([1, H * K], BF16, bq_d[:, :], "c_bq")
            bagg_sb = cload([1, V], BF16, bagg_d[:, :], "c_bagg")
            mrow_sb = cload([1, NG * GI * NTOK], BF16, mrow_d[:, :], "c_mrow")

        inp = ctx.enter_context(tc.tile_pool(name="inp", bufs=8))
        work = ctx.enter_context(tc.tile_pool(name="work", bufs=6))
        rds = ctx.enter_context(tc.tile_pool(name="rds", bufs=2))
        encp = ctx.enter_context(tc.tile_pool(name="encp", bufs=2, space="PSUM"))
        cwp = ctx.enter_context(tc.tile_pool(name="cwp", bufs=1, space="PSUM"))
        rdp = ctx.enter_context(tc.tile_pool(name="rdp", bufs=2, space="PSUM"))

        # ---- query encoder: qT_sb[k2, 4*item+h] = (query @ Wq + bq)^T ----
        qT_sb = const.tile([K, H * BL], BF16, tag="c_qT")
        qTn_sb = const.tile([K, H * BL], BF16, tag="c_qTn")
        if True:
            qenc = encp.tile([128, 512], F32, tag="enc")
            for h in range(H):
                nc.tensor.matmul(qenc[:, 128 * h:128 * (h + 1)],
                                 lhsT=wq_sb[:, 128 * h:128 * (h + 1)],
                                 rhs=qraw_sb[:], start=True, stop=not with_bias)
                if with_bias:
                    nc.tensor.matmul(qenc[:, 128 * h:128 * (h + 1)],
                                     lhsT=bq_sb[:, 128 * h:128 * (h + 1)],
                                     rhs=ones_sb[:, :BL], start=False, stop=True)
            qTr = qT_sb[:].rearrange("k (i h) -> k h i", h=H)
            for h in range(H):
                nc.vector.tensor_copy(qTr[:, h, :], qenc[:, 128 * h:128 * (h + 1)])
        nc.vector.tensor_scalar_mul(qTn_sb[:], qT_sb[:], -1.0)

        def load_w(gg):
            wbf = inp.tile([K, GI * V], BF16, tag="wbf", name=f"wbf_{gg}")
            nc.sync.dma_start(wbf[:], wbf_d[:, GI * V * gg:GI * V * (gg + 1)])
            if gg >= na:
                return None, wbf
            w_t = inp.tile([K, GI * V], F32, tag="w", name=f"w_t_{gg}")
            nc.sync.dma_start(w_t[:], w_d[:, GI * V * gg:GI * V * (gg + 1)])
            return w_t, wbf

        w_cur = load_w(0)
        for c in range(NCH):
            readT = rdp.tile([128, H * CH], F32, tag="rd")
            for g in range(GPC):
                gg = c * GPC + g
                w_nxt = load_w(gg + 1) if gg + 1 < NG else None
                w_t, wbf = w_cur
                if gg >= na:
                    # passive group: done_mask == 0 -> w_new = w; only the
                    # read path is needed, and w_new is filled host-side
                    for i in range(GI):
                        it = gg * GI + i
                        co = H * (g * GI + i)
                        nc.tensor.matmul(readT[:, co:co + H],
                                         lhsT=wbf[:, 128 * i:128 * (i + 1)],
                                         rhs=qT_sb[:, H * it:H * (it + 1)],
                                         start=True, stop=True)
                    w_cur = w_nxt
                    continue
                keyT_t = inp.tile([K, GI * NTOK], BF16, tag="keyT")
                nc.sync.dma_start(keyT_t[:], keyT_d[gg])
                valT_t = inp.tile([V, GI * NTOK], BF16, tag="valT")
                nc.sync.dma_start(valT_t[:], valT_d[gg])

                enc = encp.tile([128, 512], F32, tag="enc")
                # kenc [tok, i]
                nc.tensor.matmul(enc[:, 0:128], lhsT=keyT_t[:], rhs=wk_sb[:],
                                 start=True, stop=not with_bias)
                # kT [i, tok]
                nc.tensor.matmul(enc[:, 128:256], lhsT=wk_sb[:], rhs=keyT_t[:],
                                 start=True, stop=not with_bias)
                # venc [tok, v]
                nc.tensor.matmul(enc[:, 256:384], lhsT=valT_t[:], rhs=wv_sb[:],
                                 start=True, stop=not with_bias)
                if with_bias:
                    # masked bias: k = key_m @ Wk + m*bk  (mask folded into key on host)
                    mrow_g = mrow_sb[:, 128 * gg:128 * (gg + 1)]
                    nc.tensor.matmul(enc[:, 0:128], lhsT=mrow_g, rhs=bk_sb[:, :],
                                     start=False, stop=True)
                    nc.tensor.matmul(enc[:, 128:256], lhsT=bk_sb[:, :], rhs=mrow_g,
                                     start=False, stop=True)
                    nc.tensor.matmul(enc[:, 256:384], lhsT=ones_sb[:, :GI * NTOK],
                                     rhs=bv_sb[:, :], start=False, stop=True)

                k_sb = work.tile([GI * NTOK, K], BF16, tag="k")
                nc.scalar.copy(k_sb[:], enc[:, 0:128])
                kT_sb = work.tile([K, GI * NTOK], BF16, tag="kT")
                nc.scalar.copy(kT_sb[:], enc[:, 128:256])
                v_sb = work.tile([GI * NTOK, V], BF16, tag="v")
                nc.scalar.activation(v_sb[:], enc[:, 256:384], AF.Copy,
                                     scale=mod_sb[:, gg:gg + 1])

                # G = k @ w per item, col-tiled into partitions 32i..32i+31
                for i in range(GI):
                    nc.tensor.matmul(enc[32 * i:32 * (i + 1), 384:512],
                                     lhsT=kT_sb[:, 32 * i:32 * (i + 1)],
                                     rhs=wbf[:, 128 * i:128 * (i + 1)],
                                     start=True, stop=True, tile_position=(0, 32 * i))
                G_sb = work.tile([GI * NTOK, V], BF16, tag="G_sb")
                nc.scalar.copy(G_sb[:], enc[:, 384:512])

                # corr/rw: per-item row-tiled matmuls, one PSUM bank per item
                # (concurrent row-group matmuls must not share a bank on HW);
                # item i's bank holds corr at cols 0:128, rw at cols 128:256
                cw = cwp.tile([128, GI * 512], F32, tag="cw")
                for i in range(GI):
                    sl = slice(32 * i, 32 * (i + 1))
                    nc.tensor.matmul(cw[:, 512 * i:512 * i + 128],
                                     lhsT=k_sb[sl, :], rhs=v_sb[sl, :],
                                     start=True, stop=True, tile_position=(32 * i, 0))
                    nc.tensor.matmul(cw[:, 512 * i + 128:512 * i + 256],
                                     lhsT=k_sb[sl, :], rhs=G_sb[sl, :],
                                     start=True, stop=True, tile_position=(32 * i, 0))
                cw3 = cw[:].rearrange("p (g c) -> p g c", g=GI)

                X = work.tile([K, GI * V], BF16, tag="X")
                X3 = X[:].rearrange("p (g c) -> p g c", g=GI)
                A3 = arep_sb[:].rearrange("p (g c) -> p g c", g=GI)
                B3 = brep_sb[:].rearrange("p (g c) -> p g c", g=GI)
                nc.vector.tensor_tensor(X3, cw3[:, :, 0:128], A3, op=AOP.mult)
                U = work.tile([K, GI * V], BF16, tag="U")
                U3 = U[:].rearrange("p (g c) -> p g c", g=GI)
                nc.vector.tensor_tensor(U3, cw3[:, :, 128:256], B3, op=AOP.mult)
                wm1 = work.tile([K, GI * V], BF16, tag="wm1")
                nc.vector.tensor_scalar_sub(wm1[:], wbf[:], 1.0)
                negZ = work.tile([K, GI * V], BF16, tag="negZ")
                nc.vector.tensor_tensor(negZ[:], wm1[:], X[:], op=AOP.mult)
                Vp = work.tile([K, GI * V], BF16, tag="Vp")
                nc.gpsimd.tensor_tensor(Vp[:], negZ[:], U[:], op=AOP.add)
                wnew = work.tile([K, GI * V], F32, tag="wnew")
                nc.gpsimd.tensor_tensor(wnew[:], w_t[:], Vp[:], op=AOP.subtract)
                nc.sync.dma_start(wout_d[:, GI * V * gg:GI * V * (gg + 1)],
                                  wnew[:])

                # readT[v, 4*il+h] = (w' ^T q)[v,h] = w^T q - Vp^T q
                for i in range(GI):
                    it = gg * GI + i           # item within core
                    co = H * (g * GI + i)      # column offset within chunk
                    nc.tensor.matmul(readT[:, co:co + H],
                                     lhsT=wbf[:, 128 * i:128 * (i + 1)],
                                     rhs=qT_sb[:, H * it:H * (it + 1)],
                                     start=True, stop=False)
                    nc.tensor.matmul(readT[:, co:co + H],
                                     lhsT=Vp[:, 128 * i:128 * (i + 1)],
                                     rhs=qTn_sb[:, H * it:H * (it + 1)],
                                     start=False, stop=True)
                w_cur = w_nxt

            # ---- read aggregation for this chunk of 32 items ----
            readT_sb = rds.tile([128, H * CH], BF16, tag="rdsb")
            nc.vector.tensor_copy(readT_sb[:], readT[:])
            rr = readT_sb[:].rearrange("v (i h) -> v h i", h=H)
            aggT = rdp.tile([128, CH], F32, tag="rd")
            for h in range(H):
                nc.tensor.matmul(aggT[:, :], lhsT=wagg_sb[:, 128 * h:128 * (h + 1)],
                                 rhs=rr[:, h, :], start=(h == 0),
                                 stop=(h == H - 1) and not with_bias)
            if with_bias:
                nc.tensor.matmul(aggT[:, :], lhsT=bagg_sb[:, :],
                                 rhs=ones_sb[:, :CH], start=False, stop=True)
            aggT_sb = rds.tile([128, CH], F32, tag="aggsb")
            nc.vector.tensor_copy(aggT_sb[:], aggT[:])
            outT = rdp.tile([CH, 128], F32, tag="rd")
            nc.tensor.transpose(outT[:], aggT_sb[:], id_sb[:])
            out_sb = rds.tile([CH, 128], F32, tag="outsb")
            nc.scalar.copy(out_sb[:], outT[:])
            nc.sync.dma_start(out_d[CH * c:CH * (c + 1), :], out_sb[:])

    nc.finalize()
    return nc


def TileKernel(nc):
    return tile.TileContext(nc)


def _prep_inputs(value, key, modulation, query, w_assoc, done_mask,
                 Wk, bk, Wv, bv, A, B_mat, Wq, bq, Wagg, bagg, with_bias,
                 perms, na):
    mask = done_mask.astype(np.float32)
    key_m = (key * mask[:, None, None]).astype(np.float32)

    shared = {
        "wk": Wk.astype(NPBF),
        "wv": Wv.astype(NPBF),
        "wq": Wq.astype(NPBF),
        "wagg": np.ascontiguousarray(
            Wagg.reshape(H, K, V).transpose(1, 0, 2).reshape(V, H * V)).astype(NPBF),
        "arep": np.ascontiguousarray(np.tile(A, (1, GI))).astype(NPBF),
        "brep": np.ascontiguousarray(np.tile(B_mat, (1, GI))).astype(NPBF),
        "ident": np.eye(128, dtype=np.float32),
    }
    if with_bias:
        shared.update({
            "ones": np.ones((1, 512), NPBF),
            "bk": bk.reshape(1, K).astype(NPBF),
            "bv": bv.reshape(1, V).astype(NPBF),
            "bq": bq.reshape(1, H * K).astype(NPBF),
            "bagg": bagg.reshape(1, V).astype(NPBF),
        })

    in_maps = []
    nact = GI * na
    for c in range(NCORES):
        idx = c * BL + perms[c]
        aidx = idx[:nact]
        wperm = np.ascontiguousarray(
            w_assoc[idx].transpose(1, 0, 2).reshape(K, BL * V)).astype(np.float32)
        km = key_m[aidx].reshape(na, GI, NTOK, K)
        keyT = np.zeros((NG, K, GI * NTOK), np.float32)
        keyT[:na] = km.transpose(0, 3, 1, 2).reshape(na, K, GI * NTOK)
        valT = np.zeros((NG, V, GI * NTOK), np.float32)
        valT[:na] = value[aidx].reshape(na, GI, NTOK, V).transpose(0, 3, 1, 2).reshape(
            na, V, GI * NTOK)
        modv = np.zeros((GI * NTOK, NG), np.float32)
        modv[:, :na] = modulation[aidx, :, 0].reshape(na, GI * NTOK).T
        m = {
            "keyT": keyT.astype(NPBF),
            "valT": valT.astype(NPBF),
            "w_in": wperm,
            "w_in_bf": wperm.astype(NPBF),
            "modv": modv,
            "qraw": np.ascontiguousarray(query[idx].T).astype(NPBF),
            **shared,
        }
        if with_bias:
            mrow = mask[idx].reshape(NG * GI, 1).repeat(NTOK, axis=1)
            m["mrow"] = mrow.reshape(1, NG * GI * NTOK).astype(NPBF)
        in_maps.append(m)
    return in_maps


def kernel(value, key, modulation, query, w_assoc, done_mask,
           Wk, bk, Wv, bv, A, B_mat, Wq, bq, Wagg, bagg):
    global _last_bkr
    value = np.asarray(value, np.float32)
    key = np.asarray(key, np.float32)
    modulation = np.asarray(modulation, np.float32)
    query = np.asarray(query, np.float32)
    w_assoc = np.asarray(w_assoc, np.float32)
    done_mask_np = np.asarray(done_mask)

    with_bias = bool(np.any(bk) or np.any(bv) or np.any(bq) or np.any(bagg))

    maskf = (done_mask_np != 0)
    perms = []
    max_active = 0
    for c in range(NCORES):
        mc = maskf[c * BL:(c + 1) * BL]
        perm = np.argsort(~mc, kind="stable")
        perms.append(perm)
        max_active = max(max_active, int(mc.sum()))
    na = min(NG, (max_active + GI - 1) // GI)

    key_prog = (with_bias, na)
    if key_prog not in _prog_cache:
        _prog_cache[key_prog] = _build_program(with_bias, na)
    nc = _prog_cache[key_prog]

    in_maps = _prep_inputs(value, key, modulation, query, w_assoc, done_mask_np,
                           np.asarray(Wk), np.asarray(bk), np.asarray(Wv),
                           np.asarray(bv), np.asarray(A), np.asarray(B_mat),
                           np.asarray(Wq), np.asarray(bq), np.asarray(Wagg),
                           np.asarray(bagg), with_bias, perms, na)

    _ensure_axon_hooks()
    try:
        bkr = run_bass_kernel_spmd(nc, in_maps, list(range(NCORES)))
    except ModuleNotFoundError:
        os.environ["BASS_NEVER_TRACE"] = "1"
        bkr = run_bass_kernel_spmd(nc, in_maps, list(range(NCORES)))
    _last_bkr = bkr

    w_new = np.empty((B, K, V), np.float32)
    out = np.empty((B, V), np.float32)
    nact = GI * na
    for c in range(NCORES):
        idx = c * BL + perms[c]
        wc = bkr.results[c]["w_out"].reshape(K, BL, V).transpose(1, 0, 2)
        w_new[idx[:nact]] = wc[:nact]
        w_new[idx[nact:]] = w_assoc[idx[nact:]]
        out[idx] = bkr.results[c]["outp"]
    return out, w_new


# revision 28
# speedup vs baseline: 1.0700x; 1.0133x over previous
"""Trainium2 Bass kernel for the Hebbian scatter-memory module.

Strategy: pure data-parallel over batch (1024 items -> 8 cores x 128 items).
Per core, items are processed in groups of 4 (4 x 32 tokens = 128 partitions).

Math per item b (reference):
  k    = key[b] @ Wk + bk                      [32,128]
  v    = (value[b] @ Wv + bv) * mod[b]         [32,128]
  corr = k^T v ; reg = k^T k
  dw   = A*(1-w)*corr - B*(reg @ w)
  w'   = w + m[b]*dw
  q    = (query[b] @ Wq + bq) -> [4,128]
  out  = (q @ w').flat @ Wagg + bagg

Device mapping (all matmuls bf16 inputs, fp32 PSUM accumulate):
  - host folds done_mask into key (m^2 == m for binary masks), transposes
    key/value to feature-major [K, tok] so no on-chip transposes are needed
  - reg @ w is computed associatively as k^T (k @ w) via the feature-major
    encoder output kT, avoiding a [128,512] PSUM->SBUF copy of reg
  - elementwise chain (5 ops): X = corr*A ; negZ = (w-1)*X ;
    U = rw*B ; Vp = negZ + U (= -dw) ; w' = w - Vp
  - read: w'^T q = w^T q + Vp^T (-q), two accumulated matmuls per item,
    so no bf16 cast of w' is needed
"""

import os
from contextlib import ExitStack

import numpy as np

import concourse.bass as bass
import concourse.bacc as bacc
import concourse.mybir as mybir
import concourse.tile as tile
from concourse.bass_utils import run_bass_kernel_spmd

NCORES = 8
B = 1024
NTOK = 32
K = 128
V = 128
H = 4
BL = B // NCORES          # items per core
GI = 4                    # items per group (4*32 tokens = 128)
NG = BL // GI             # groups per core
CH = 32                   # items per read-aggregation chunk
NCH = BL // CH            # chunks per core
GPC = CH // GI            # groups per chunk

F32 = mybir.dt.float32
BF16 = mybir.dt.bfloat16
NPBF = mybir.dt.np(BF16)

AOP = mybir.AluOpType
AF = mybir.ActivationFunctionType

_prog_cache = {}
_last_bkr = None  # BassKernelResults of the most recent run (for test harness)


def _ensure_axon_hooks():
    """Provide antenv.axon_hooks if the image lacks it (needed only when
    BASS_TRACE profiling is requested; inert otherwise)."""
    try:
        import antenv.axon_hooks  # noqa: F401
        return
    except ImportError:
        pass
    import types
    import ctypes
    import contextlib
    import sys

    mod = types.ModuleType("antenv.axon_hooks")
    holder = {"h": None}
    mod.set_axon_ntff_profile_hook = lambda h: holder.__setitem__("h", h)
    mod.get_axon_ntff_profile_hook = lambda: holder["h"]

    so = "/opt/axon/libaxon_pjrt.so"
    if os.path.exists(so):
        try:
            lib = ctypes.CDLL(so)
            if hasattr(lib, "axon_start_nrt_profile"):
                lib.axon_start_nrt_profile.argtypes = [
                    ctypes.POINTER(ctypes.c_int64), ctypes.c_size_t]
                lib.axon_start_nrt_profile.restype = ctypes.c_int64
                lib.axon_stop_nrt_profile.argtypes = [ctypes.c_char_p]
                lib.axon_stop_nrt_profile.restype = ctypes.c_int64

                @contextlib.contextmanager
                def _hook(output_dir, device_ids):
                    import jax
                    jax.devices()
                    if device_ids:
                        ids = (ctypes.c_int64 * len(device_ids))(*device_ids)
                        rc = lib.axon_start_nrt_profile(ids, len(device_ids))
                    else:
                        rc = lib.axon_start_nrt_profile(None, 0)
                    if rc != 0:
                        raise RuntimeError(f"axon_start_nrt_profile rc={rc}")
                    try:
                        yield
                    finally:
                        n = lib.axon_stop_nrt_profile(str(output_dir).encode())
                        print(f"profile: {n} file(s) written to {output_dir}")

                holder["h"] = _hook
        except Exception:
            pass

    import antenv
    antenv.axon_hooks = mod
    sys.modules["antenv.axon_hooks"] = mod


def _active_positions(na):
    if na >= NG:
        return list(range(NG))
    return sorted({round(i * NG / na) for i in range(na)} if na else set())


def _build_program(with_bias, na):
    apos = _active_positions(na)
    # even spread can collide on rounding; repair to exactly na slots
    apos = list(apos)
    extra = [s for s in range(NG) if s not in set(apos)]
    while len(apos) < na:
        apos.append(extra.pop(0))
    apos = sorted(apos[:na])
    aset = set(apos)
    nc = bacc.Bacc()

    keyT_d = nc.dram_tensor("keyT", [NG, K, GI * NTOK], BF16, kind="ExternalInput")
    valT_d = nc.dram_tensor("valT", [NG, V, GI * NTOK], BF16, kind="ExternalInput")
    w_d = nc.dram_tensor("w_in", [K, BL * V], F32, kind="ExternalInput")
    wbf_d = nc.dram_tensor("w_in_bf", [K, BL * V], BF16, kind="ExternalInput")
    mod_d = nc.dram_tensor("modv", [GI * NTOK, NG], F32, kind="ExternalInput")
    qraw_d = nc.dram_tensor("qraw", [K, BL], BF16, kind="ExternalInput")
    wk_d = nc.dram_tensor("wk", [K, K], BF16, kind="ExternalInput")
    wv_d = nc.dram_tensor("wv", [V, V], BF16, kind="ExternalInput")
    wq_d = nc.dram_tensor("wq", [K, H * K], BF16, kind="ExternalInput")
    wagg_d = nc.dram_tensor("wagg", [V, H * V], BF16, kind="ExternalInput")
    arep_d = nc.dram_tensor("arep", [K, GI * V], BF16, kind="ExternalInput")
    brep_d = nc.dram_tensor("brep", [K, GI * V], BF16, kind="ExternalInput")
    id_d = nc.dram_tensor("ident", [128, 128], F32, kind="ExternalInput")
    if with_bias:
        ones_d = nc.dram_tensor("ones", [1, 512], BF16, kind="ExternalInput")
        bk_d = nc.dram_tensor("bk", [1, K], BF16, kind="ExternalInput")
        bv_d = nc.dram_tensor("bv", [1, V], BF16, kind="ExternalInput")
        bq_d = nc.dram_tensor("bq", [1, H * K], BF16, kind="ExternalInput")
        bagg_d = nc.dram_tensor("bagg", [1, V], BF16, kind="ExternalInput")
        mrow_d = nc.dram_tensor("mrow", [1, NG * GI * NTOK], BF16, kind="ExternalInput")

    wout_d = nc.dram_tensor("w_out", [K, BL * V], F32, kind="ExternalOutput")
    out_d = nc.dram_tensor("outp", [BL, V], F32, kind="ExternalOutput")

    with TileKernel(nc) as tc, ExitStack() as ctx:
        const = ctx.enter_context(tc.tile_pool(name="const", bufs=1))

        def cload(shape, dtype, src, tag):
            t = const.tile(shape, dtype, tag=tag)
            nc.sync.dma_start(t[:], src)
            return t

        wk_sb = cload([K, K], BF16, wk_d[:, :], "c_wk")
        wv_sb = cload([V, V], BF16, wv_d[:, :], "c_wv")
        wq_sb = cload([K, H * K], BF16, wq_d[:, :], "c_wq")
        # [v, (h, o)] layout so wagg_sb[:, 128h:+128] is lhsT for head h
        wagg_sb = cload([V, H * V], BF16, wagg_d[:, :], "c_wagg")
        arep_sb = cload([K, GI * V], BF16, arep_d[:, :], "c_arep")
        brep_sb = cload([K, GI * V], BF16, brep_d[:, :], "c_brep")
        id_sb = cload([128, 128], F32, id_d[:, :], "c_id")
        mod_sb = cload([GI * NTOK, NG], F32, mod_d[:, :], "c_mod")
        qraw_sb = cload([K, BL], BF16, qraw_d[:, :], "c_qraw")
        if with_bias:
            ones_sb = cload([1, 512], BF16, ones_d[:, :], "c_ones")
            bk_sb = cload([1, K], BF16, bk_d[:, :], "c_bk")
            bv_sb = cload([1, V], BF16, bv_d[:, :], "c_bv")
            bq_sb = cload([1, H * K], BF16, bq_d[:, :], "c_bq")
            bagg_sb = cload([1, V], BF16, bagg_d[:, :], "c_bagg")
            mrow_sb = cload([1, NG * GI * NTOK], BF16, mrow_d[:, :], "c_mrow")

        inp = ctx.enter_context(tc.tile_pool(name="inp", bufs=8))
        work = ctx.enter_context(tc.tile_pool(name="work", bufs=6))
        rds = ctx.enter_context(tc.tile_pool(name="rds", bufs=2))
        encp = ctx.enter_context(tc.tile_pool(name="encp", bufs=2, space="PSUM"))
        cwp = ctx.enter_context(tc.tile_pool(name="cwp", bufs=1, space="PSUM"))
        rdp = ctx.enter_context(tc.tile_pool(name="rdp", bufs=2, space="PSUM"))

        # ---- query encoder: qT_sb[k2, 4*item+h] = (query @ Wq + bq)^T ----
        qT_sb = const.tile([K, H * BL], BF16, tag="c_qT")
        qTn_sb = const.tile([K, H * BL], BF16, tag="c_qTn")
        if True:
            qenc = encp.tile([128, 512], F32, tag="enc")
            for h in range(H):
                nc.tensor.matmul(qenc[:, 128 * h:128 * (h + 1)],
                                 lhsT=wq_sb[:, 128 * h:128 * (h + 1)],
                                 rhs=qraw_sb[:], start=True, stop=not with_bias)
                if with_bias:
                    nc.tensor.matmul(qenc[:, 128 * h:128 * (h + 1)],
                                     lhsT=bq_sb[:, 128 * h:128 * (h + 1)],
                                     rhs=ones_sb[:, :BL], start=False, stop=True)
            qTr = qT_sb[:].rearrange("k (i h) -> k h i", h=H)
            for h in range(H):
                nc.vector.tensor_copy(qTr[:, h, :], qenc[:, 128 * h:128 * (h + 1)])
        nc.vector.tensor_scalar_mul(qTn_sb[:], qT_sb[:], -1.0)

        def load_w(gg):
            wbf = inp.tile([K, GI * V], BF16, tag="wbf", name=f"wbf_{gg}")
            nc.sync.dma_start(wbf[:], wbf_d[:, GI * V * gg:GI * V * (gg + 1)])
            if gg not in aset:
                return None, wbf
            w_t = inp.tile([K, GI * V], F32, tag="w", name=f"w_t_{gg}")
            nc.sync.dma_start(w_t[:], w_d[:, GI * V * gg:GI * V * (gg + 1)])
            return w_t, wbf

        w_cur = load_w(0)
        act_idx = {}
        for i, s in enumerate(apos):
            act_idx[s] = i
        for c in range(NCH):
            readT = rdp.tile([128, H * CH], F32, tag="rd")
            for g in range(GPC):
                gg = c * GPC + g
                w_nxt = load_w(gg + 1) if gg + 1 < NG else None
                w_t, wbf = w_cur
                if gg not in aset:
                    # passive group: done_mask == 0 -> w_new = w; only the
                    # read path is needed, and w_new is filled host-side
                    for i in range(GI):
                        it = gg * GI + i
                        co = H * (g * GI + i)
                        nc.tensor.matmul(readT[:, co:co + H],
                                         lhsT=wbf[:, 128 * i:128 * (i + 1)],
                                         rhs=qT_sb[:, H * it:H * (it + 1)],
                                         start=True, stop=True)
                    w_cur = w_nxt
                    continue
                ai = act_idx[gg]
                keyT_t = inp.tile([K, GI * NTOK], BF16, tag="keyT")
                nc.sync.dma_start(keyT_t[:], keyT_d[ai])
                valT_t = inp.tile([V, GI * NTOK], BF16, tag="valT")
                nc.sync.dma_start(valT_t[:], valT_d[ai])

                enc = encp.tile([128, 512], F32, tag="enc")
                # kenc [tok, i]
                nc.tensor.matmul(enc[:, 0:128], lhsT=keyT_t[:], rhs=wk_sb[:],
                                 start=True, stop=not with_bias)
                # kT [i, tok]
                nc.tensor.matmul(enc[:, 128:256], lhsT=wk_sb[:], rhs=keyT_t[:],
                                 start=True, stop=not with_bias)
                # venc [tok, v]
                nc.tensor.matmul(enc[:, 256:384], lhsT=valT_t[:], rhs=wv_sb[:],
                                 start=True, stop=not with_bias)
                if with_bias:
                    # masked bias: k = key_m @ Wk + m*bk  (mask folded into key on host)
                    mrow_g = mrow_sb[:, 128 * gg:128 * (gg + 1)]
                    nc.tensor.matmul(enc[:, 0:128], lhsT=mrow_g, rhs=bk_sb[:, :],
                                     start=False, stop=True)
                    nc.tensor.matmul(enc[:, 128:256], lhsT=bk_sb[:, :], rhs=mrow_g,
                                     start=False, stop=True)
                    nc.tensor.matmul(enc[:, 256:384], lhsT=ones_sb[:, :GI * NTOK],
                                     rhs=bv_sb[:, :], start=False, stop=True)

                k_sb = work.tile([GI * NTOK, K], BF16, tag="k")
                nc.scalar.copy(k_sb[:], enc[:, 0:128])
                kT_sb = work.tile([K, GI * NTOK], BF16, tag="kT")
                nc.scalar.copy(kT_sb[:], enc[:, 128:256])
                v_sb = work.tile([GI * NTOK, V], BF16, tag="v")
                nc.scalar.activation(v_sb[:], enc[:, 256:384], AF.Copy,
                                     scale=mod_sb[:, ai:ai + 1])

                # G = k @ w per item, col-tiled into partitions 32i..32i+31
                for i in range(GI):
                    nc.tensor.matmul(enc[32 * i:32 * (i + 1), 384:512],
                                     lhsT=kT_sb[:, 32 * i:32 * (i + 1)],
                                     rhs=wbf[:, 128 * i:128 * (i + 1)],
                                     start=True, stop=True, tile_position=(0, 32 * i))
                G_sb = work.tile([GI * NTOK, V], BF16, tag="G_sb")
                nc.scalar.copy(G_sb[:], enc[:, 384:512])

                # corr/rw: per-item row-tiled matmuls, one PSUM bank per item
                # (concurrent row-group matmuls must not share a bank on HW);
                # item i's bank holds corr at cols 0:128, rw at cols 128:256
                cw = cwp.tile([128, GI * 512], F32, tag="cw")
                for i in range(GI):
                    sl = slice(32 * i, 32 * (i + 1))
                    nc.tensor.matmul(cw[:, 512 * i:512 * i + 128],
                                     lhsT=k_sb[sl, :], rhs=v_sb[sl, :],
                                     start=True, stop=True, tile_position=(32 * i, 0))
                    nc.tensor.matmul(cw[:, 512 * i + 128:512 * i + 256],
                                     lhsT=k_sb[sl, :], rhs=G_sb[sl, :],
                                     start=True, stop=True, tile_position=(32 * i, 0))
                cw3 = cw[:].rearrange("p (g c) -> p g c", g=GI)

                X = work.tile([K, GI * V], BF16, tag="X")
                X3 = X[:].rearrange("p (g c) -> p g c", g=GI)
                A3 = arep_sb[:].rearrange("p (g c) -> p g c", g=GI)
                B3 = brep_sb[:].rearrange("p (g c) -> p g c", g=GI)
                nc.vector.tensor_tensor(X3, cw3[:, :, 0:128], A3, op=AOP.mult)
                U = work.tile([K, GI * V], BF16, tag="U")
                U3 = U[:].rearrange("p (g c) -> p g c", g=GI)
                nc.vector.tensor_tensor(U3, cw3[:, :, 128:256], B3, op=AOP.mult)
                wm1 = work.tile([K, GI * V], BF16, tag="wm1")
                nc.vector.tensor_scalar_sub(wm1[:], wbf[:], 1.0)
                negZ = work.tile([K, GI * V], BF16, tag="negZ")
                nc.vector.tensor_tensor(negZ[:], wm1[:], X[:], op=AOP.mult)
                Vp = work.tile([K, GI * V], BF16, tag="Vp")
                nc.gpsimd.tensor_tensor(Vp[:], negZ[:], U[:], op=AOP.add)
                wnew = work.tile([K, GI * V], F32, tag="wnew")
                nc.gpsimd.tensor_tensor(wnew[:], w_t[:], Vp[:], op=AOP.subtract)
                nc.sync.dma_start(wout_d[:, GI * V * gg:GI * V * (gg + 1)],
                                  wnew[:])

                # readT[v, 4*il+h] = (w' ^T q)[v,h] = w^T q - Vp^T q
                for i in range(GI):
                    it = gg * GI + i           # item within core
                    co = H * (g * GI + i)      # column offset within chunk
                    nc.tensor.matmul(readT[:, co:co + H],
                                     lhsT=wbf[:, 128 * i:128 * (i + 1)],
                                     rhs=qT_sb[:, H * it:H * (it + 1)],
                                     start=True, stop=False)
                    nc.tensor.matmul(readT[:, co:co + H],
                                     lhsT=Vp[:, 128 * i:128 * (i + 1)],
                                     rhs=qTn_sb[:, H * it:H * (it + 1)],
                                     start=False, stop=True)
                w_cur = w_nxt

            # ---- read aggregation for this chunk of 32 items ----
            readT_sb = rds.tile([128, H * CH], BF16, tag="rdsb")
            nc.vector.tensor_copy(readT_sb[:], readT[:])
            rr = readT_sb[:].rearrange("v (i h) -> v h i", h=H)
            aggT = rdp.tile([128, CH], F32, tag="rd")
            for h in range(H):
                nc.tensor.matmul(aggT[:, :], lhsT=wagg_sb[:, 128 * h:128 * (h + 1)],
                                 rhs=rr[:, h, :], start=(h == 0),
                                 stop=(h == H - 1) and not with_bias)
            if with_bias:
                nc.tensor.matmul(aggT[:, :], lhsT=bagg_sb[:, :],
                                 rhs=ones_sb[:, :CH], start=False, stop=True)
            aggT_sb = rds.tile([128, CH], F32, tag="aggsb")
            nc.vector.tensor_copy(aggT_sb[:], aggT[:])
            outT = rdp.tile([CH, 128], F32, tag="rd")
            nc.tensor.transpose(outT[:], aggT_sb[:], id_sb[:])
            out_sb = rds.tile([CH, 128], F32, tag="outsb")
            nc.scalar.copy(out_sb[:], outT[:])
            nc.sync.dma_start(out_d[CH * c:CH * (c + 1), :], out_sb[:])

    nc.finalize()
    return nc


def TileKernel(nc):
    return tile.TileContext(nc)


def _prep_inputs(value, key, modulation, query, w_assoc, done_mask,
                 Wk, bk, Wv, bv, A, B_mat, Wq, bq, Wagg, bagg, with_bias,
                 perms, na, aset):
    mask = done_mask.astype(np.float32)
    key_m = (key * mask[:, None, None]).astype(np.float32)

    shared = {
        "wk": Wk.astype(NPBF),
        "wv": Wv.astype(NPBF),
        "wq": Wq.astype(NPBF),
        "wagg": np.ascontiguousarray(
            Wagg.reshape(H, K, V).transpose(1, 0, 2).reshape(V, H * V)).astype(NPBF),
        "arep": np.ascontiguousarray(np.tile(A, (1, GI))).astype(NPBF),
        "brep": np.ascontiguousarray(np.tile(B_mat, (1, GI))).astype(NPBF),
        "ident": np.eye(128, dtype=np.float32),
    }
    if with_bias:
        shared.update({
            "ones": np.ones((1, 512), NPBF),
            "bk": bk.reshape(1, K).astype(NPBF),
            "bv": bv.reshape(1, V).astype(NPBF),
            "bq": bq.reshape(1, H * K).astype(NPBF),
            "bagg": bagg.reshape(1, V).astype(NPBF),
        })

    in_maps = []
    alist = sorted(aset)
    for c in range(NCORES):
        idx = c * BL + perms[c]
        arows = np.concatenate([np.arange(GI * s, GI * (s + 1)) for s in alist])
        aidx = idx[arows]
        wperm = np.ascontiguousarray(
            w_assoc[idx].transpose(1, 0, 2).reshape(K, BL * V)).astype(np.float32)
        nal = len(alist)
        km = key_m[aidx].reshape(nal, GI, NTOK, K)
        keyT = np.zeros((NG, K, GI * NTOK), np.float32)
        keyT[:nal] = km.transpose(0, 3, 1, 2).reshape(nal, K, GI * NTOK)
        valT = np.zeros((NG, V, GI * NTOK), np.float32)
        valT[:nal] = value[aidx].reshape(nal, GI, NTOK, V).transpose(0, 3, 1, 2).reshape(
            nal, V, GI * NTOK)
        modv = np.zeros((GI * NTOK, NG), np.float32)
        modv[:, :len(alist)] = modulation[aidx, :, 0].reshape(len(alist), GI * NTOK).T
        m = {
            "keyT": keyT.astype(NPBF),
            "valT": valT.astype(NPBF),
            "w_in": wperm,
            "w_in_bf": wperm.astype(NPBF),
            "modv": modv,
            "qraw": np.ascontiguousarray(query[idx].T).astype(NPBF),
            **shared,
        }
        if with_bias:
            mrow = mask[idx].reshape(NG * GI, 1).repeat(NTOK, axis=1)
            m["mrow"] = mrow.reshape(1, NG * GI * NTOK).astype(NPBF)
        in_maps.append(m)
    return in_maps


def kernel(value, key, modulation, query, w_assoc, done_mask,
           Wk, bk, Wv, bv, A, B_mat, Wq, bq, Wagg, bagg):
    global _last_bkr
    value = np.asarray(value, np.float32)
    key = np.asarray(key, np.float32)
    modulation = np.asarray(modulation, np.float32)
    query = np.asarray(query, np.float32)
    w_assoc = np.asarray(w_assoc, np.float32)
    done_mask_np = np.asarray(done_mask)

    with_bias = bool(np.any(bk) or np.any(bv) or np.any(bq) or np.any(bagg))

    maskf = (done_mask_np != 0)
    max_active = 0
    for c in range(NCORES):
        mc = maskf[c * BL:(c + 1) * BL]
        max_active = max(max_active, int(mc.sum()))
    na = min(NG, (max_active + GI - 1) // GI)
    apos = _active_positions(na)
    apos = list(apos)
    extra = [s for s in range(NG) if s not in set(apos)]
    while len(apos) < na:
        apos.append(extra.pop(0))
    aset = set(sorted(apos[:na]))
    perms = []
    slot_active_rows = np.zeros(BL, bool)
    for s in range(NG):
        if s in aset:
            slot_active_rows[GI * s:GI * (s + 1)] = True
    for c in range(NCORES):
        mc = maskf[c * BL:(c + 1) * BL]
        act = list(np.nonzero(mc)[0])
        pas = list(np.nonzero(~mc)[0])
        order = []
        for s in range(NG):
            src_q = act if s in aset else pas
            for _ in range(GI):
                if src_q is act and not act:
                    src_q = pas
                order.append(src_q.pop(0))
        perms.append(np.array(order))

    key_prog = (with_bias, na)
    if key_prog not in _prog_cache:
        _prog_cache[key_prog] = _build_program(with_bias, na)
    nc = _prog_cache[key_prog]

    in_maps = _prep_inputs(value, key, modulation, query, w_assoc, done_mask_np,
                           np.asarray(Wk), np.asarray(bk), np.asarray(Wv),
                           np.asarray(bv), np.asarray(A), np.asarray(B_mat),
                           np.asarray(Wq), np.asarray(bq), np.asarray(Wagg),
                           np.asarray(bagg), with_bias, perms, na, aset)

    _ensure_axon_hooks()
    try:
        bkr = run_bass_kernel_spmd(nc, in_maps, list(range(NCORES)))
    except ModuleNotFoundError:
        os.environ["BASS_NEVER_TRACE"] = "1"
        bkr = run_bass_kernel_spmd(nc, in_maps, list(range(NCORES)))
    _last_bkr = bkr

    w_new = np.empty((B, K, V), np.float32)
    out = np.empty((B, V), np.float32)
    sar = np.zeros(BL, bool)
    for s in aset:
        sar[GI * s:GI * (s + 1)] = True
    for c in range(NCORES):
        idx = c * BL + perms[c]
        wc = bkr.results[c]["w_out"].reshape(K, BL, V).transpose(1, 0, 2)
        w_new[idx[sar]] = wc[sar]
        w_new[idx[~sar]] = w_assoc[idx[~sar]]
        out[idx] = bkr.results[c]["outp"]
    return out, w_new


# revision 29
# speedup vs baseline: 1.3146x; 1.2287x over previous
"""Trainium2 Bass kernel for the Hebbian scatter-memory module.

Strategy: pure data-parallel over batch (1024 items -> 8 cores x 128 items).
Per core, items are processed in groups of 4 (4 x 32 tokens = 128 partitions).

Math per item b (reference):
  k    = key[b] @ Wk + bk                      [32,128]
  v    = (value[b] @ Wv + bv) * mod[b]         [32,128]
  corr = k^T v ; reg = k^T k
  dw   = A*(1-w)*corr - B*(reg @ w)
  w'   = w + m[b]*dw
  q    = (query[b] @ Wq + bq) -> [4,128]
  out  = (q @ w').flat @ Wagg + bagg

Device mapping (all matmuls bf16 inputs, fp32 PSUM accumulate):
  - host folds done_mask into key (m^2 == m for binary masks), transposes
    key/value to feature-major [K, tok] so no on-chip transposes are needed
  - reg @ w is computed associatively as k^T (k @ w) via the feature-major
    encoder output kT, avoiding a [128,512] PSUM->SBUF copy of reg
  - elementwise chain (5 ops): X = corr*A ; negZ = (w-1)*X ;
    U = rw*B ; Vp = negZ + U (= -dw) ; w' = w - Vp
  - read: w'^T q = w^T q + Vp^T (-q), two accumulated matmuls per item,
    so no bf16 cast of w' is needed
"""

import os
from contextlib import ExitStack

import numpy as np

import concourse.bass as bass
import concourse.bacc as bacc
import concourse.mybir as mybir
import concourse.tile as tile
from concourse.bass_utils import run_bass_kernel_spmd

NCORES = 8
B = 1024
NTOK = 32
K = 128
V = 128
H = 4
BL = B // NCORES          # items per core
GI = 4                    # items per group (4*32 tokens = 128)
NG = BL // GI             # groups per core
CH = 32                   # items per read-aggregation chunk
NCH = BL // CH            # chunks per core
GPC = CH // GI            # groups per chunk

F32 = mybir.dt.float32
BF16 = mybir.dt.bfloat16
NPBF = mybir.dt.np(BF16)

AOP = mybir.AluOpType
AF = mybir.ActivationFunctionType

_prog_cache = {}
_last_bkr = None  # BassKernelResults of the most recent run (for test harness)


def _ensure_axon_hooks():
    """Provide antenv.axon_hooks if the image lacks it (needed only when
    BASS_TRACE profiling is requested; inert otherwise)."""
    try:
        import antenv.axon_hooks  # noqa: F401
        return
    except ImportError:
        pass
    import types
    import ctypes
    import contextlib
    import sys

    mod = types.ModuleType("antenv.axon_hooks")
    holder = {"h": None}
    mod.set_axon_ntff_profile_hook = lambda h: holder.__setitem__("h", h)
    mod.get_axon_ntff_profile_hook = lambda: holder["h"]

    so = "/opt/axon/libaxon_pjrt.so"
    if os.path.exists(so):
        try:
            lib = ctypes.CDLL(so)
            if hasattr(lib, "axon_start_nrt_profile"):
                lib.axon_start_nrt_profile.argtypes = [
                    ctypes.POINTER(ctypes.c_int64), ctypes.c_size_t]
                lib.axon_start_nrt_profile.restype = ctypes.c_int64
                lib.axon_stop_nrt_profile.argtypes = [ctypes.c_char_p]
                lib.axon_stop_nrt_profile.restype = ctypes.c_int64

                @contextlib.contextmanager
                def _hook(output_dir, device_ids):
                    import jax
                    jax.devices()
                    if device_ids:
                        ids = (ctypes.c_int64 * len(device_ids))(*device_ids)
                        rc = lib.axon_start_nrt_profile(ids, len(device_ids))
                    else:
                        rc = lib.axon_start_nrt_profile(None, 0)
                    if rc != 0:
                        raise RuntimeError(f"axon_start_nrt_profile rc={rc}")
                    try:
                        yield
                    finally:
                        n = lib.axon_stop_nrt_profile(str(output_dir).encode())
                        print(f"profile: {n} file(s) written to {output_dir}")

                holder["h"] = _hook
        except Exception:
            pass

    import antenv
    antenv.axon_hooks = mod
    sys.modules["antenv.axon_hooks"] = mod


def _active_positions(na):
    if na >= NG:
        return list(range(NG))
    return sorted({round(i * NG / na) for i in range(na)} if na else set())


def _build_program(with_bias, na):
    apos = _active_positions(na)
    # even spread can collide on rounding; repair to exactly na slots
    apos = list(apos)
    extra = [s for s in range(NG) if s not in set(apos)]
    while len(apos) < na:
        apos.append(extra.pop(0))
    apos = sorted(apos[:na])
    aset = set(apos)
    nc = bacc.Bacc()

    kv_d = nc.dram_tensor("kv", [NG, K, 2 * GI * NTOK], BF16, kind="ExternalInput")
    w_d = nc.dram_tensor("w_in", [K, BL * V], F32, kind="ExternalInput")
    wbf_d = nc.dram_tensor("w_in_bf", [K, BL * V], BF16, kind="ExternalInput")
    mod_d = nc.dram_tensor("modv", [GI * NTOK, NG], F32, kind="ExternalInput")
    qraw_d = nc.dram_tensor("qraw", [K, BL], BF16, kind="ExternalInput")
    wk_d = nc.dram_tensor("wk", [K, K], BF16, kind="ExternalInput")
    wv_d = nc.dram_tensor("wv", [V, V], BF16, kind="ExternalInput")
    wq_d = nc.dram_tensor("wq", [K, H * K], BF16, kind="ExternalInput")
    wagg_d = nc.dram_tensor("wagg", [V, H * V], BF16, kind="ExternalInput")
    arep_d = nc.dram_tensor("arep", [K, GI * V], BF16, kind="ExternalInput")
    brep_d = nc.dram_tensor("brep", [K, GI * V], BF16, kind="ExternalInput")
    id_d = nc.dram_tensor("ident", [128, 128], F32, kind="ExternalInput")
    if with_bias:
        ones_d = nc.dram_tensor("ones", [1, 512], BF16, kind="ExternalInput")
        bk_d = nc.dram_tensor("bk", [1, K], BF16, kind="ExternalInput")
        bv_d = nc.dram_tensor("bv", [1, V], BF16, kind="ExternalInput")
        bq_d = nc.dram_tensor("bq", [1, H * K], BF16, kind="ExternalInput")
        bagg_d = nc.dram_tensor("bagg", [1, V], BF16, kind="ExternalInput")
        mrow_d = nc.dram_tensor("mrow", [1, NG * GI * NTOK], BF16, kind="ExternalInput")

    wout_d = nc.dram_tensor("w_out", [K, BL * V], F32, kind="ExternalOutput")
    out_d = nc.dram_tensor("outp", [BL, V], F32, kind="ExternalOutput")

    with TileKernel(nc) as tc, ExitStack() as ctx:
        const = ctx.enter_context(tc.tile_pool(name="const", bufs=1))

        def cload(shape, dtype, src, tag):
            t = const.tile(shape, dtype, tag=tag)
            nc.sync.dma_start(t[:], src)
            return t

        wk_sb = cload([K, K], BF16, wk_d[:, :], "c_wk")
        wv_sb = cload([V, V], BF16, wv_d[:, :], "c_wv")
        wq_sb = cload([K, H * K], BF16, wq_d[:, :], "c_wq")
        # [v, (h, o)] layout so wagg_sb[:, 128h:+128] is lhsT for head h
        wagg_sb = cload([V, H * V], BF16, wagg_d[:, :], "c_wagg")
        arep_sb = cload([K, GI * V], BF16, arep_d[:, :], "c_arep")
        brep_sb = cload([K, GI * V], BF16, brep_d[:, :], "c_brep")
        id_sb = cload([128, 128], F32, id_d[:, :], "c_id")
        mod_sb = cload([GI * NTOK, NG], F32, mod_d[:, :], "c_mod")
        qraw_sb = cload([K, BL], BF16, qraw_d[:, :], "c_qraw")
        if with_bias:
            ones_sb = cload([1, 512], BF16, ones_d[:, :], "c_ones")
            bk_sb = cload([1, K], BF16, bk_d[:, :], "c_bk")
            bv_sb = cload([1, V], BF16, bv_d[:, :], "c_bv")
            bq_sb = cload([1, H * K], BF16, bq_d[:, :], "c_bq")
            bagg_sb = cload([1, V], BF16, bagg_d[:, :], "c_bagg")
            mrow_sb = cload([1, NG * GI * NTOK], BF16, mrow_d[:, :], "c_mrow")

        inp = ctx.enter_context(tc.tile_pool(name="inp", bufs=8))
        work = ctx.enter_context(tc.tile_pool(name="work", bufs=6))
        rds = ctx.enter_context(tc.tile_pool(name="rds", bufs=2))
        encp = ctx.enter_context(tc.tile_pool(name="encp", bufs=2, space="PSUM"))
        cwp = ctx.enter_context(tc.tile_pool(name="cwp", bufs=1, space="PSUM"))
        rdp = ctx.enter_context(tc.tile_pool(name="rdp", bufs=2, space="PSUM"))

        # ---- query encoder: qT_sb[k2, 4*item+h] = (query @ Wq + bq)^T ----
        qT_sb = const.tile([K, H * BL], BF16, tag="c_qT")
        qTn_sb = const.tile([K, H * BL], BF16, tag="c_qTn")
        if True:
            qenc = encp.tile([128, 512], F32, tag="enc")
            for h in range(H):
                nc.tensor.matmul(qenc[:, 128 * h:128 * (h + 1)],
                                 lhsT=wq_sb[:, 128 * h:128 * (h + 1)],
                                 rhs=qraw_sb[:], start=True, stop=not with_bias)
                if with_bias:
                    nc.tensor.matmul(qenc[:, 128 * h:128 * (h + 1)],
                                     lhsT=bq_sb[:, 128 * h:128 * (h + 1)],
                                     rhs=ones_sb[:, :BL], start=False, stop=True)
            qTr = qT_sb[:].rearrange("k (i h) -> k h i", h=H)
            for h in range(H):
                nc.vector.tensor_copy(qTr[:, h, :], qenc[:, 128 * h:128 * (h + 1)])
        nc.vector.tensor_scalar_mul(qTn_sb[:], qT_sb[:], -1.0)

        def load_w(gg):
            wbf = inp.tile([K, GI * V], BF16, tag="wbf", name=f"wbf_{gg}")
            nc.scalar.dma_start(wbf[:], wbf_d[:, GI * V * gg:GI * V * (gg + 1)])
            if gg not in aset:
                return None, wbf
            w_t = inp.tile([K, GI * V], F32, tag="w", name=f"w_t_{gg}")
            nc.sync.dma_start(w_t[:], w_d[:, GI * V * gg:GI * V * (gg + 1)])
            return w_t, wbf

        w_cur = load_w(0)
        act_idx = {}
        for i, s in enumerate(apos):
            act_idx[s] = i
        for c in range(NCH):
            readT = rdp.tile([128, H * CH], F32, tag="rd")
            for g in range(GPC):
                gg = c * GPC + g
                w_nxt = load_w(gg + 1) if gg + 1 < NG else None
                w_t, wbf = w_cur
                if gg not in aset:
                    # passive group: done_mask == 0 -> w_new = w; only the
                    # read path is needed, and w_new is filled host-side
                    for i in range(GI):
                        it = gg * GI + i
                        co = H * (g * GI + i)
                        nc.tensor.matmul(readT[:, co:co + H],
                                         lhsT=wbf[:, 128 * i:128 * (i + 1)],
                                         rhs=qT_sb[:, H * it:H * (it + 1)],
                                         start=True, stop=True)
                    w_cur = w_nxt
                    continue
                ai = act_idx[gg]
                kv_t = inp.tile([K, 2 * GI * NTOK], BF16, tag="kv")
                nc.scalar.dma_start(kv_t[:], kv_d[ai])
                keyT_t = kv_t[:, 0:GI * NTOK]
                valT_t = kv_t[:, GI * NTOK:2 * GI * NTOK]

                enc = encp.tile([128, 512], F32, tag="enc")
                # kenc [tok, i]
                nc.tensor.matmul(enc[:, 0:128], lhsT=keyT_t, rhs=wk_sb[:],
                                 start=True, stop=not with_bias)
                # kT [i, tok]
                nc.tensor.matmul(enc[:, 128:256], lhsT=wk_sb[:], rhs=keyT_t,
                                 start=True, stop=not with_bias)
                # venc [tok, v]
                nc.tensor.matmul(enc[:, 256:384], lhsT=valT_t, rhs=wv_sb[:],
                                 start=True, stop=not with_bias)
                if with_bias:
                    # masked bias: k = key_m @ Wk + m*bk  (mask folded into key on host)
                    mrow_g = mrow_sb[:, 128 * gg:128 * (gg + 1)]
                    nc.tensor.matmul(enc[:, 0:128], lhsT=mrow_g, rhs=bk_sb[:, :],
                                     start=False, stop=True)
                    nc.tensor.matmul(enc[:, 128:256], lhsT=bk_sb[:, :], rhs=mrow_g,
                                     start=False, stop=True)
                    nc.tensor.matmul(enc[:, 256:384], lhsT=ones_sb[:, :GI * NTOK],
                                     rhs=bv_sb[:, :], start=False, stop=True)

                k_sb = work.tile([GI * NTOK, K], BF16, tag="k")
                nc.scalar.copy(k_sb[:], enc[:, 0:128])
                kT_sb = work.tile([K, GI * NTOK], BF16, tag="kT")
                nc.scalar.copy(kT_sb[:], enc[:, 128:256])
                v_sb = work.tile([GI * NTOK, V], BF16, tag="v")
                nc.scalar.activation(v_sb[:], enc[:, 256:384], AF.Copy,
                                     scale=mod_sb[:, ai:ai + 1])

                # G = k @ w per item, col-tiled into partitions 32i..32i+31
                for i in range(GI):
                    nc.tensor.matmul(enc[32 * i:32 * (i + 1), 384:512],
                                     lhsT=kT_sb[:, 32 * i:32 * (i + 1)],
                                     rhs=wbf[:, 128 * i:128 * (i + 1)],
                                     start=True, stop=True, tile_position=(0, 32 * i))
                G_sb = work.tile([GI * NTOK, V], BF16, tag="G_sb")
                nc.scalar.copy(G_sb[:], enc[:, 384:512])

                # corr/rw: per-item row-tiled matmuls, one PSUM bank per item
                # (concurrent row-group matmuls must not share a bank on HW);
                # item i's bank holds corr at cols 0:128, rw at cols 128:256
                cw = cwp.tile([128, GI * 512], F32, tag="cw")
                for i in range(GI):
                    sl = slice(32 * i, 32 * (i + 1))
                    nc.tensor.matmul(cw[:, 512 * i:512 * i + 128],
                                     lhsT=k_sb[sl, :], rhs=v_sb[sl, :],
                                     start=True, stop=True, tile_position=(32 * i, 0))
                    nc.tensor.matmul(cw[:, 512 * i + 128:512 * i + 256],
                                     lhsT=k_sb[sl, :], rhs=G_sb[sl, :],
                                     start=True, stop=True, tile_position=(32 * i, 0))
                cw3 = cw[:].rearrange("p (g c) -> p g c", g=GI)

                X = work.tile([K, GI * V], BF16, tag="X")
                X3 = X[:].rearrange("p (g c) -> p g c", g=GI)
                A3 = arep_sb[:].rearrange("p (g c) -> p g c", g=GI)
                B3 = brep_sb[:].rearrange("p (g c) -> p g c", g=GI)
                nc.vector.tensor_tensor(X3, cw3[:, :, 0:128], A3, op=AOP.mult)
                U = work.tile([K, GI * V], BF16, tag="U")
                U3 = U[:].rearrange("p (g c) -> p g c", g=GI)
                nc.vector.tensor_tensor(U3, cw3[:, :, 128:256], B3, op=AOP.mult)
                wm1 = work.tile([K, GI * V], BF16, tag="wm1")
                nc.vector.tensor_scalar_sub(wm1[:], wbf[:], 1.0)
                negZ = work.tile([K, GI * V], BF16, tag="negZ")
                nc.vector.tensor_tensor(negZ[:], wm1[:], X[:], op=AOP.mult)
                Vp = work.tile([K, GI * V], BF16, tag="Vp")
                nc.gpsimd.tensor_tensor(Vp[:], negZ[:], U[:], op=AOP.add)
                wnew = work.tile([K, GI * V], F32, tag="wnew")
                nc.gpsimd.tensor_tensor(wnew[:], w_t[:], Vp[:], op=AOP.subtract)
                nc.sync.dma_start(wout_d[:, GI * V * gg:GI * V * (gg + 1)],
                                  wnew[:])

                # readT[v, 4*il+h] = (w' ^T q)[v,h] = w^T q - Vp^T q
                for i in range(GI):
                    it = gg * GI + i           # item within core
                    co = H * (g * GI + i)      # column offset within chunk
                    nc.tensor.matmul(readT[:, co:co + H],
                                     lhsT=wbf[:, 128 * i:128 * (i + 1)],
                                     rhs=qT_sb[:, H * it:H * (it + 1)],
                                     start=True, stop=False)
                    nc.tensor.matmul(readT[:, co:co + H],
                                     lhsT=Vp[:, 128 * i:128 * (i + 1)],
                                     rhs=qTn_sb[:, H * it:H * (it + 1)],
                                     start=False, stop=True)
                w_cur = w_nxt

            # ---- read aggregation for this chunk of 32 items ----
            readT_sb = rds.tile([128, H * CH], BF16, tag="rdsb")
            nc.vector.tensor_copy(readT_sb[:], readT[:])
            rr = readT_sb[:].rearrange("v (i h) -> v h i", h=H)
            aggT = rdp.tile([128, CH], F32, tag="rd")
            for h in range(H):
                nc.tensor.matmul(aggT[:, :], lhsT=wagg_sb[:, 128 * h:128 * (h + 1)],
                                 rhs=rr[:, h, :], start=(h == 0),
                                 stop=(h == H - 1) and not with_bias)
            if with_bias:
                nc.tensor.matmul(aggT[:, :], lhsT=bagg_sb[:, :],
                                 rhs=ones_sb[:, :CH], start=False, stop=True)
            aggT_sb = rds.tile([128, CH], F32, tag="aggsb")
            nc.vector.tensor_copy(aggT_sb[:], aggT[:])
            outT = rdp.tile([CH, 128], F32, tag="rd")
            nc.tensor.transpose(outT[:], aggT_sb[:], id_sb[:])
            out_sb = rds.tile([CH, 128], F32, tag="outsb")
            nc.scalar.copy(out_sb[:], outT[:])
            nc.sync.dma_start(out_d[CH * c:CH * (c + 1), :], out_sb[:])

    nc.finalize()
    return nc


def TileKernel(nc):
    return tile.TileContext(nc)


def _prep_inputs(value, key, modulation, query, w_assoc, done_mask,
                 Wk, bk, Wv, bv, A, B_mat, Wq, bq, Wagg, bagg, with_bias,
                 perms, na, aset):
    mask = done_mask.astype(np.float32)
    key_m = (key * mask[:, None, None]).astype(np.float32)

    shared = {
        "wk": Wk.astype(NPBF),
        "wv": Wv.astype(NPBF),
        "wq": Wq.astype(NPBF),
        "wagg": np.ascontiguousarray(
            Wagg.reshape(H, K, V).transpose(1, 0, 2).reshape(V, H * V)).astype(NPBF),
        "arep": np.ascontiguousarray(np.tile(A, (1, GI))).astype(NPBF),
        "brep": np.ascontiguousarray(np.tile(B_mat, (1, GI))).astype(NPBF),
        "ident": np.eye(128, dtype=np.float32),
    }
    if with_bias:
        shared.update({
            "ones": np.ones((1, 512), NPBF),
            "bk": bk.reshape(1, K).astype(NPBF),
            "bv": bv.reshape(1, V).astype(NPBF),
            "bq": bq.reshape(1, H * K).astype(NPBF),
            "bagg": bagg.reshape(1, V).astype(NPBF),
        })

    in_maps = []
    alist = sorted(aset)
    for c in range(NCORES):
        idx = c * BL + perms[c]
        arows = np.concatenate([np.arange(GI * s, GI * (s + 1)) for s in alist])
        aidx = idx[arows]
        wperm = np.ascontiguousarray(
            w_assoc[idx].transpose(1, 0, 2).reshape(K, BL * V)).astype(np.float32)
        nal = len(alist)
        km = key_m[aidx].reshape(nal, GI, NTOK, K)
        kv = np.zeros((NG, K, 2 * GI * NTOK), np.float32)
        kv[:nal, :, :GI * NTOK] = km.transpose(0, 3, 1, 2).reshape(nal, K, GI * NTOK)
        kv[:nal, :, GI * NTOK:] = value[aidx].reshape(
            nal, GI, NTOK, V).transpose(0, 3, 1, 2).reshape(nal, V, GI * NTOK)
        modv = np.zeros((GI * NTOK, NG), np.float32)
        modv[:, :len(alist)] = modulation[aidx, :, 0].reshape(len(alist), GI * NTOK).T
        m = {
            "kv": kv.astype(NPBF),
            "w_in": wperm,
            "w_in_bf": wperm.astype(NPBF),
            "modv": modv,
            "qraw": np.ascontiguousarray(query[idx].T).astype(NPBF),
            **shared,
        }
        if with_bias:
            mrow = mask[idx].reshape(NG * GI, 1).repeat(NTOK, axis=1)
            m["mrow"] = mrow.reshape(1, NG * GI * NTOK).astype(NPBF)
        in_maps.append(m)
    return in_maps


def kernel(value, key, modulation, query, w_assoc, done_mask,
           Wk, bk, Wv, bv, A, B_mat, Wq, bq, Wagg, bagg):
    global _last_bkr
    value = np.asarray(value, np.float32)
    key = np.asarray(key, np.float32)
    modulation = np.asarray(modulation, np.float32)
    query = np.asarray(query, np.float32)
    w_assoc = np.asarray(w_assoc, np.float32)
    done_mask_np = np.asarray(done_mask)

    with_bias = bool(np.any(bk) or np.any(bv) or np.any(bq) or np.any(bagg))

    maskf = (done_mask_np != 0)
    max_active = 0
    for c in range(NCORES):
        mc = maskf[c * BL:(c + 1) * BL]
        max_active = max(max_active, int(mc.sum()))
    na = min(NG, (max_active + GI - 1) // GI)
    apos = _active_positions(na)
    apos = list(apos)
    extra = [s for s in range(NG) if s not in set(apos)]
    while len(apos) < na:
        apos.append(extra.pop(0))
    aset = set(sorted(apos[:na]))
    perms = []
    slot_active_rows = np.zeros(BL, bool)
    for s in range(NG):
        if s in aset:
            slot_active_rows[GI * s:GI * (s + 1)] = True
    for c in range(NCORES):
        mc = maskf[c * BL:(c + 1) * BL]
        act = list(np.nonzero(mc)[0])
        pas = list(np.nonzero(~mc)[0])
        order = []
        for s in range(NG):
            src_q = act if s in aset else pas
            for _ in range(GI):
                if src_q is act and not act:
                    src_q = pas
                order.append(src_q.pop(0))
        perms.append(np.array(order))

    key_prog = (with_bias, na)
    if key_prog not in _prog_cache:
        _prog_cache[key_prog] = _build_program(with_bias, na)
    nc = _prog_cache[key_prog]

    in_maps = _prep_inputs(value, key, modulation, query, w_assoc, done_mask_np,
                           np.asarray(Wk), np.asarray(bk), np.asarray(Wv),
                           np.asarray(bv), np.asarray(A), np.asarray(B_mat),
                           np.asarray(Wq), np.asarray(bq), np.asarray(Wagg),
                           np.asarray(bagg), with_bias, perms, na, aset)

    _ensure_axon_hooks()
    try:
        bkr = run_bass_kernel_spmd(nc, in_maps, list(range(NCORES)))
    except ModuleNotFoundError:
        os.environ["BASS_NEVER_TRACE"] = "1"
        bkr = run_bass_kernel_spmd(nc, in_maps, list(range(NCORES)))
    _last_bkr = bkr

    w_new = np.empty((B, K, V), np.float32)
    out = np.empty((B, V), np.float32)
    sar = np.zeros(BL, bool)
    for s in aset:
        sar[GI * s:GI * (s + 1)] = True
    for c in range(NCORES):
        idx = c * BL + perms[c]
        wc = bkr.results[c]["w_out"].reshape(K, BL, V).transpose(1, 0, 2)
        w_new[idx[sar]] = wc[sar]
        w_new[idx[~sar]] = w_assoc[idx[~sar]]
        out[idx] = bkr.results[c]["outp"]
    return out, w_new


# revision 30
# speedup vs baseline: 1.3286x; 1.0106x over previous
"""Trainium2 Bass kernel for the Hebbian scatter-memory module.

Strategy: pure data-parallel over batch (1024 items -> 8 cores x 128 items).
Per core, items are processed in groups of 4 (4 x 32 tokens = 128 partitions).

Math per item b (reference):
  k    = key[b] @ Wk + bk                      [32,128]
  v    = (value[b] @ Wv + bv) * mod[b]         [32,128]
  corr = k^T v ; reg = k^T k
  dw   = A*(1-w)*corr - B*(reg @ w)
  w'   = w + m[b]*dw
  q    = (query[b] @ Wq + bq) -> [4,128]
  out  = (q @ w').flat @ Wagg + bagg

Device mapping (all matmuls bf16 inputs, fp32 PSUM accumulate):
  - host folds done_mask into key (m^2 == m for binary masks), transposes
    key/value to feature-major [K, tok] so no on-chip transposes are needed
  - reg @ w is computed associatively as k^T (k @ w) via the feature-major
    encoder output kT, avoiding a [128,512] PSUM->SBUF copy of reg
  - elementwise chain (5 ops): X = corr*A ; negZ = (w-1)*X ;
    U = rw*B ; Vp = negZ + U (= -dw) ; w' = w - Vp
  - read: w'^T q = w^T q + Vp^T (-q), two accumulated matmuls per item,
    so no bf16 cast of w' is needed
"""

import os
from contextlib import ExitStack

import numpy as np

import concourse.bass as bass
import concourse.bacc as bacc
import concourse.mybir as mybir
import concourse.tile as tile
from concourse.bass_utils import run_bass_kernel_spmd

NCORES = 8
B = 1024
NTOK = 32
K = 128
V = 128
H = 4
BL = B // NCORES          # items per core
GI = 4                    # items per group (4*32 tokens = 128)
NG = BL // GI             # groups per core
CH = 32                   # items per read-aggregation chunk
NCH = BL // CH            # chunks per core
GPC = CH // GI            # groups per chunk

F32 = mybir.dt.float32
BF16 = mybir.dt.bfloat16
NPBF = mybir.dt.np(BF16)

AOP = mybir.AluOpType
AF = mybir.ActivationFunctionType

_prog_cache = {}
_last_bkr = None  # BassKernelResults of the most recent run (for test harness)


def _ensure_axon_hooks():
    """Provide antenv.axon_hooks if the image lacks it (needed only when
    BASS_TRACE profiling is requested; inert otherwise)."""
    try:
        import antenv.axon_hooks  # noqa: F401
        return
    except ImportError:
        pass
    import types
    import ctypes
    import contextlib
    import sys

    mod = types.ModuleType("antenv.axon_hooks")
    holder = {"h": None}
    mod.set_axon_ntff_profile_hook = lambda h: holder.__setitem__("h", h)
    mod.get_axon_ntff_profile_hook = lambda: holder["h"]

    so = "/opt/axon/libaxon_pjrt.so"
    if os.path.exists(so):
        try:
            lib = ctypes.CDLL(so)
            if hasattr(lib, "axon_start_nrt_profile"):
                lib.axon_start_nrt_profile.argtypes = [
                    ctypes.POINTER(ctypes.c_int64), ctypes.c_size_t]
                lib.axon_start_nrt_profile.restype = ctypes.c_int64
                lib.axon_stop_nrt_profile.argtypes = [ctypes.c_char_p]
                lib.axon_stop_nrt_profile.restype = ctypes.c_int64

                @contextlib.contextmanager
                def _hook(output_dir, device_ids):
                    import jax
                    jax.devices()
                    if device_ids:
                        ids = (ctypes.c_int64 * len(device_ids))(*device_ids)
                        rc = lib.axon_start_nrt_profile(ids, len(device_ids))
                    else:
                        rc = lib.axon_start_nrt_profile(None, 0)
                    if rc != 0:
                        raise RuntimeError(f"axon_start_nrt_profile rc={rc}")
                    try:
                        yield
                    finally:
                        n = lib.axon_stop_nrt_profile(str(output_dir).encode())
                        print(f"profile: {n} file(s) written to {output_dir}")

                holder["h"] = _hook
        except Exception:
            pass

    import antenv
    antenv.axon_hooks = mod
    sys.modules["antenv.axon_hooks"] = mod


def _active_positions(na):
    if na >= NG:
        return list(range(NG))
    return sorted({round(i * NG / na) for i in range(na)} if na else set())


def _build_program(with_bias, na):
    apos = _active_positions(na)
    # even spread can collide on rounding; repair to exactly na slots
    apos = list(apos)
    extra = [s for s in range(NG) if s not in set(apos)]
    while len(apos) < na:
        apos.append(extra.pop(0))
    apos = sorted(apos[:na])
    aset = set(apos)
    nc = bacc.Bacc()

    kv_d = nc.dram_tensor("kv", [NG, K, 2 * GI * NTOK], BF16, kind="ExternalInput")
    w_d = nc.dram_tensor("w_in", [K, BL * V], F32, kind="ExternalInput")
    wbf_d = nc.dram_tensor("w_in_bf", [K, BL * V], BF16, kind="ExternalInput")
    mod_d = nc.dram_tensor("modv", [GI * NTOK, NG], F32, kind="ExternalInput")
    qraw_d = nc.dram_tensor("qraw", [K, BL], BF16, kind="ExternalInput")
    wk_d = nc.dram_tensor("wk", [K, K], BF16, kind="ExternalInput")
    wv_d = nc.dram_tensor("wv", [V, V], BF16, kind="ExternalInput")
    wq_d = nc.dram_tensor("wq", [K, H * K], BF16, kind="ExternalInput")
    wagg_d = nc.dram_tensor("wagg", [V, H * V], BF16, kind="ExternalInput")
    arep_d = nc.dram_tensor("arep", [K, GI * V], BF16, kind="ExternalInput")
    brep_d = nc.dram_tensor("brep", [K, GI * V], BF16, kind="ExternalInput")
    id_d = nc.dram_tensor("ident", [128, 128], F32, kind="ExternalInput")
    if with_bias:
        ones_d = nc.dram_tensor("ones", [1, 512], BF16, kind="ExternalInput")
        bk_d = nc.dram_tensor("bk", [1, K], BF16, kind="ExternalInput")
        bv_d = nc.dram_tensor("bv", [1, V], BF16, kind="ExternalInput")
        bq_d = nc.dram_tensor("bq", [1, H * K], BF16, kind="ExternalInput")
        bagg_d = nc.dram_tensor("bagg", [1, V], BF16, kind="ExternalInput")
        mrow_d = nc.dram_tensor("mrow", [1, NG * GI * NTOK], BF16, kind="ExternalInput")

    wout_d = nc.dram_tensor("w_out", [K, BL * V], F32, kind="ExternalOutput")
    out_d = nc.dram_tensor("outp", [BL, V], F32, kind="ExternalOutput")

    with TileKernel(nc) as tc, ExitStack() as ctx:
        const = ctx.enter_context(tc.tile_pool(name="const", bufs=1))

        def cload(shape, dtype, src, tag):
            t = const.tile(shape, dtype, tag=tag)
            nc.sync.dma_start(t[:], src)
            return t

        wk_sb = cload([K, K], BF16, wk_d[:, :], "c_wk")
        wv_sb = cload([V, V], BF16, wv_d[:, :], "c_wv")
        wq_sb = cload([K, H * K], BF16, wq_d[:, :], "c_wq")
        # [v, (h, o)] layout so wagg_sb[:, 128h:+128] is lhsT for head h
        wagg_sb = cload([V, H * V], BF16, wagg_d[:, :], "c_wagg")
        arep_sb = cload([K, GI * V], BF16, arep_d[:, :], "c_arep")
        brep_sb = cload([K, GI * V], BF16, brep_d[:, :], "c_brep")
        id_sb = cload([128, 128], F32, id_d[:, :], "c_id")
        mod_sb = cload([GI * NTOK, NG], F32, mod_d[:, :], "c_mod")
        qraw_sb = cload([K, BL], BF16, qraw_d[:, :], "c_qraw")
        if with_bias:
            ones_sb = cload([1, 512], BF16, ones_d[:, :], "c_ones")
            bk_sb = cload([1, K], BF16, bk_d[:, :], "c_bk")
            bv_sb = cload([1, V], BF16, bv_d[:, :], "c_bv")
            bq_sb = cload([1, H * K], BF16, bq_d[:, :], "c_bq")
            bagg_sb = cload([1, V], BF16, bagg_d[:, :], "c_bagg")
            mrow_sb = cload([1, NG * GI * NTOK], BF16, mrow_d[:, :], "c_mrow")

        inp = ctx.enter_context(tc.tile_pool(name="inp", bufs=8))
        work = ctx.enter_context(tc.tile_pool(name="work", bufs=6))
        rds = ctx.enter_context(tc.tile_pool(name="rds", bufs=2))
        encp = ctx.enter_context(tc.tile_pool(name="encp", bufs=2, space="PSUM"))
        cwpa = ctx.enter_context(tc.tile_pool(name="cwpa", bufs=1, space="PSUM"))
        cwpb = ctx.enter_context(tc.tile_pool(name="cwpb", bufs=1, space="PSUM"))
        rdp = ctx.enter_context(tc.tile_pool(name="rdp", bufs=2, space="PSUM"))

        # ---- query encoder: qT_sb[k2, 4*item+h] = (query @ Wq + bq)^T ----
        qT_sb = const.tile([K, H * BL], BF16, tag="c_qT")
        qTn_sb = const.tile([K, H * BL], BF16, tag="c_qTn")
        if True:
            qenc = encp.tile([128, 512], F32, tag="enc")
            for h in range(H):
                nc.tensor.matmul(qenc[:, 128 * h:128 * (h + 1)],
                                 lhsT=wq_sb[:, 128 * h:128 * (h + 1)],
                                 rhs=qraw_sb[:], start=True, stop=not with_bias)
                if with_bias:
                    nc.tensor.matmul(qenc[:, 128 * h:128 * (h + 1)],
                                     lhsT=bq_sb[:, 128 * h:128 * (h + 1)],
                                     rhs=ones_sb[:, :BL], start=False, stop=True)
            qTr = qT_sb[:].rearrange("k (i h) -> k h i", h=H)
            for h in range(H):
                nc.vector.tensor_copy(qTr[:, h, :], qenc[:, 128 * h:128 * (h + 1)])
        nc.vector.tensor_scalar_mul(qTn_sb[:], qT_sb[:], -1.0)

        def load_w(gg):
            wbf = inp.tile([K, GI * V], BF16, tag="wbf", name=f"wbf_{gg}")
            nc.scalar.dma_start(wbf[:], wbf_d[:, GI * V * gg:GI * V * (gg + 1)])
            if gg not in aset:
                return None, wbf
            w_t = inp.tile([K, GI * V], F32, tag="w", name=f"w_t_{gg}")
            nc.sync.dma_start(w_t[:], w_d[:, GI * V * gg:GI * V * (gg + 1)])
            return w_t, wbf

        w_cur = load_w(0)
        act_idx = {}
        for i, s in enumerate(apos):
            act_idx[s] = i
        for c in range(NCH):
            readT = rdp.tile([128, H * CH], F32, tag="rd")
            for g in range(GPC):
                gg = c * GPC + g
                w_nxt = load_w(gg + 1) if gg + 1 < NG else None
                w_t, wbf = w_cur
                if gg not in aset:
                    # passive group: done_mask == 0 -> w_new = w; only the
                    # read path is needed, and w_new is filled host-side
                    for i in range(GI):
                        it = gg * GI + i
                        co = H * (g * GI + i)
                        nc.tensor.matmul(readT[:, co:co + H],
                                         lhsT=wbf[:, 128 * i:128 * (i + 1)],
                                         rhs=qT_sb[:, H * it:H * (it + 1)],
                                         start=True, stop=True)
                    w_cur = w_nxt
                    continue
                ai = act_idx[gg]
                kv_t = inp.tile([K, 2 * GI * NTOK], BF16, tag="kv")
                nc.scalar.dma_start(kv_t[:], kv_d[ai])
                keyT_t = kv_t[:, 0:GI * NTOK]
                valT_t = kv_t[:, GI * NTOK:2 * GI * NTOK]

                enc = encp.tile([128, 512], F32, tag="enc")
                # kenc [tok, i]
                nc.tensor.matmul(enc[:, 0:128], lhsT=keyT_t, rhs=wk_sb[:],
                                 start=True, stop=not with_bias)
                # kT [i, tok]
                nc.tensor.matmul(enc[:, 128:256], lhsT=wk_sb[:], rhs=keyT_t,
                                 start=True, stop=not with_bias)
                # venc [tok, v]
                nc.tensor.matmul(enc[:, 256:384], lhsT=valT_t, rhs=wv_sb[:],
                                 start=True, stop=not with_bias)
                if with_bias:
                    # masked bias: k = key_m @ Wk + m*bk  (mask folded into key on host)
                    mrow_g = mrow_sb[:, 128 * gg:128 * (gg + 1)]
                    nc.tensor.matmul(enc[:, 0:128], lhsT=mrow_g, rhs=bk_sb[:, :],
                                     start=False, stop=True)
                    nc.tensor.matmul(enc[:, 128:256], lhsT=bk_sb[:, :], rhs=mrow_g,
                                     start=False, stop=True)
                    nc.tensor.matmul(enc[:, 256:384], lhsT=ones_sb[:, :GI * NTOK],
                                     rhs=bv_sb[:, :], start=False, stop=True)

                k_sb = work.tile([GI * NTOK, K], BF16, tag="k")
                nc.scalar.copy(k_sb[:], enc[:, 0:128])
                kT_sb = work.tile([K, GI * NTOK], BF16, tag="kT")
                nc.scalar.copy(kT_sb[:], enc[:, 128:256])
                v_sb = work.tile([GI * NTOK, V], BF16, tag="v")
                nc.scalar.activation(v_sb[:], enc[:, 256:384], AF.Copy,
                                     scale=mod_sb[:, ai:ai + 1])

                # G = k @ w per item, col-tiled into partitions 32i..32i+31
                for i in range(GI):
                    nc.tensor.matmul(enc[32 * i:32 * (i + 1), 384:512],
                                     lhsT=kT_sb[:, 32 * i:32 * (i + 1)],
                                     rhs=wbf[:, 128 * i:128 * (i + 1)],
                                     start=True, stop=True, tile_position=(0, 32 * i))
                G_sb = work.tile([GI * NTOK, V], BF16, tag="G_sb")
                nc.scalar.copy(G_sb[:], enc[:, 384:512])

                # corr/rw: per-item row-tiled matmuls, one PSUM bank per item
                # (concurrent row-group matmuls must not share a bank on HW);
                # item i's bank holds corr at cols 0:128, rw at cols 128:256.
                # Two half-group tiles so X/U of the first half release their
                # banks while the second half is still in the matmul stage.
                cwa = cwpa.tile([128, 2 * 512], F32, tag="cwa")
                cwb = cwpb.tile([128, 2 * 512], F32, tag="cwb")
                halves = (cwa, cwb)
                for i in range(GI):
                    sl = slice(32 * i, 32 * (i + 1))
                    cw = halves[i // 2]
                    o = 512 * (i % 2)
                    nc.tensor.matmul(cw[:, o:o + 128],
                                     lhsT=k_sb[sl, :], rhs=v_sb[sl, :],
                                     start=True, stop=True, tile_position=(32 * i, 0))
                    nc.tensor.matmul(cw[:, o + 128:o + 256],
                                     lhsT=k_sb[sl, :], rhs=G_sb[sl, :],
                                     start=True, stop=True, tile_position=(32 * i, 0))

                X = work.tile([K, GI * V], BF16, tag="X")
                U = work.tile([K, GI * V], BF16, tag="U")
                A3 = arep_sb[:].rearrange("p (g c) -> p g c", g=GI)
                B3 = brep_sb[:].rearrange("p (g c) -> p g c", g=GI)
                X3 = X[:].rearrange("p (g c) -> p g c", g=GI)
                U3 = U[:].rearrange("p (g c) -> p g c", g=GI)
                for h2, cw in enumerate(halves):
                    cw3 = cw[:].rearrange("p (g c) -> p g c", g=2)
                    gs = slice(2 * h2, 2 * h2 + 2)
                    nc.vector.tensor_tensor(X3[:, gs, :], cw3[:, :, 0:128], A3[:, gs, :],
                                            op=AOP.mult)
                    nc.vector.tensor_tensor(U3[:, gs, :], cw3[:, :, 128:256], B3[:, gs, :],
                                            op=AOP.mult)
                wm1 = work.tile([K, GI * V], BF16, tag="wm1")
                nc.vector.tensor_scalar_sub(wm1[:], wbf[:], 1.0)
                negZ = work.tile([K, GI * V], BF16, tag="negZ")
                nc.vector.tensor_tensor(negZ[:], wm1[:], X[:], op=AOP.mult)
                Vp = work.tile([K, GI * V], BF16, tag="Vp")
                nc.gpsimd.tensor_tensor(Vp[:], negZ[:], U[:], op=AOP.add)
                wnew = work.tile([K, GI * V], F32, tag="wnew")
                nc.gpsimd.tensor_tensor(wnew[:], w_t[:], Vp[:], op=AOP.subtract)
                nc.sync.dma_start(wout_d[:, GI * V * gg:GI * V * (gg + 1)],
                                  wnew[:])

                # readT[v, 4*il+h] = (w' ^T q)[v,h] = w^T q - Vp^T q
                for i in range(GI):
                    it = gg * GI + i           # item within core
                    co = H * (g * GI + i)      # column offset within chunk
                    nc.tensor.matmul(readT[:, co:co + H],
                                     lhsT=wbf[:, 128 * i:128 * (i + 1)],
                                     rhs=qT_sb[:, H * it:H * (it + 1)],
                                     start=True, stop=False)
                    nc.tensor.matmul(readT[:, co:co + H],
                                     lhsT=Vp[:, 128 * i:128 * (i + 1)],
                                     rhs=qTn_sb[:, H * it:H * (it + 1)],
                                     start=False, stop=True)
                w_cur = w_nxt

            # ---- read aggregation for this chunk of 32 items ----
            readT_sb = rds.tile([128, H * CH], BF16, tag="rdsb")
            nc.vector.tensor_copy(readT_sb[:], readT[:])
            rr = readT_sb[:].rearrange("v (i h) -> v h i", h=H)
            aggT = rdp.tile([128, CH], F32, tag="rd")
            for h in range(H):
                nc.tensor.matmul(aggT[:, :], lhsT=wagg_sb[:, 128 * h:128 * (h + 1)],
                                 rhs=rr[:, h, :], start=(h == 0),
                                 stop=(h == H - 1) and not with_bias)
            if with_bias:
                nc.tensor.matmul(aggT[:, :], lhsT=bagg_sb[:, :],
                                 rhs=ones_sb[:, :CH], start=False, stop=True)
            aggT_sb = rds.tile([128, CH], F32, tag="aggsb")
            nc.vector.tensor_copy(aggT_sb[:], aggT[:])
            outT = rdp.tile([CH, 128], F32, tag="rd")
            nc.tensor.transpose(outT[:], aggT_sb[:], id_sb[:])
            out_sb = rds.tile([CH, 128], F32, tag="outsb")
            nc.scalar.copy(out_sb[:], outT[:])
            nc.sync.dma_start(out_d[CH * c:CH * (c + 1), :], out_sb[:])

    nc.finalize()
    return nc


def TileKernel(nc):
    return tile.TileContext(nc)


def _prep_inputs(value, key, modulation, query, w_assoc, done_mask,
                 Wk, bk, Wv, bv, A, B_mat, Wq, bq, Wagg, bagg, with_bias,
                 perms, na, aset):
    mask = done_mask.astype(np.float32)
    key_m = (key * mask[:, None, None]).astype(np.float32)

    shared = {
        "wk": Wk.astype(NPBF),
        "wv": Wv.astype(NPBF),
        "wq": Wq.astype(NPBF),
        "wagg": np.ascontiguousarray(
            Wagg.reshape(H, K, V).transpose(1, 0, 2).reshape(V, H * V)).astype(NPBF),
        "arep": np.ascontiguousarray(np.tile(A, (1, GI))).astype(NPBF),
        "brep": np.ascontiguousarray(np.tile(B_mat, (1, GI))).astype(NPBF),
        "ident": np.eye(128, dtype=np.float32),
    }
    if with_bias:
        shared.update({
            "ones": np.ones((1, 512), NPBF),
            "bk": bk.reshape(1, K).astype(NPBF),
            "bv": bv.reshape(1, V).astype(NPBF),
            "bq": bq.reshape(1, H * K).astype(NPBF),
            "bagg": bagg.reshape(1, V).astype(NPBF),
        })

    in_maps = []
    alist = sorted(aset)
    for c in range(NCORES):
        idx = c * BL + perms[c]
        arows = np.concatenate([np.arange(GI * s, GI * (s + 1)) for s in alist])
        aidx = idx[arows]
        wperm = np.ascontiguousarray(
            w_assoc[idx].transpose(1, 0, 2).reshape(K, BL * V)).astype(np.float32)
        nal = len(alist)
        km = key_m[aidx].reshape(nal, GI, NTOK, K)
        kv = np.zeros((NG, K, 2 * GI * NTOK), np.float32)
        kv[:nal, :, :GI * NTOK] = km.transpose(0, 3, 1, 2).reshape(nal, K, GI * NTOK)
        kv[:nal, :, GI * NTOK:] = value[aidx].reshape(
            nal, GI, NTOK, V).transpose(0, 3, 1, 2).reshape(nal, V, GI * NTOK)
        modv = np.zeros((GI * NTOK, NG), np.float32)
        modv[:, :len(alist)] = modulation[aidx, :, 0].reshape(len(alist), GI * NTOK).T
        m = {
            "kv": kv.astype(NPBF),
            "w_in": wperm,
            "w_in_bf": wperm.astype(NPBF),
            "modv": modv,
            "qraw": np.ascontiguousarray(query[idx].T).astype(NPBF),
            **shared,
        }
        if with_bias:
            mrow = mask[idx].reshape(NG * GI, 1).repeat(NTOK, axis=1)
            m["mrow"] = mrow.reshape(1, NG * GI * NTOK).astype(NPBF)
        in_maps.append(m)
    return in_maps


def kernel(value, key, modulation, query, w_assoc, done_mask,
           Wk, bk, Wv, bv, A, B_mat, Wq, bq, Wagg, bagg):
    global _last_bkr
    value = np.asarray(value, np.float32)
    key = np.asarray(key, np.float32)
    modulation = np.asarray(modulation, np.float32)
    query = np.asarray(query, np.float32)
    w_assoc = np.asarray(w_assoc, np.float32)
    done_mask_np = np.asarray(done_mask)

    with_bias = bool(np.any(bk) or np.any(bv) or np.any(bq) or np.any(bagg))

    maskf = (done_mask_np != 0)
    max_active = 0
    for c in range(NCORES):
        mc = maskf[c * BL:(c + 1) * BL]
        max_active = max(max_active, int(mc.sum()))
    na = min(NG, (max_active + GI - 1) // GI)
    apos = _active_positions(na)
    apos = list(apos)
    extra = [s for s in range(NG) if s not in set(apos)]
    while len(apos) < na:
        apos.append(extra.pop(0))
    aset = set(sorted(apos[:na]))
    perms = []
    slot_active_rows = np.zeros(BL, bool)
    for s in range(NG):
        if s in aset:
            slot_active_rows[GI * s:GI * (s + 1)] = True
    for c in range(NCORES):
        mc = maskf[c * BL:(c + 1) * BL]
        act = list(np.nonzero(mc)[0])
        pas = list(np.nonzero(~mc)[0])
        order = []
        for s in range(NG):
            src_q = act if s in aset else pas
            for _ in range(GI):
                if src_q is act and not act:
                    src_q = pas
                order.append(src_q.pop(0))
        perms.append(np.array(order))

    key_prog = (with_bias, na)
    if key_prog not in _prog_cache:
        _prog_cache[key_prog] = _build_program(with_bias, na)
    nc = _prog_cache[key_prog]

    in_maps = _prep_inputs(value, key, modulation, query, w_assoc, done_mask_np,
                           np.asarray(Wk), np.asarray(bk), np.asarray(Wv),
                           np.asarray(bv), np.asarray(A), np.asarray(B_mat),
                           np.asarray(Wq), np.asarray(bq), np.asarray(Wagg),
                           np.asarray(bagg), with_bias, perms, na, aset)

    _ensure_axon_hooks()
    try:
        bkr = run_bass_kernel_spmd(nc, in_maps, list(range(NCORES)))
    except ModuleNotFoundError:
        os.environ["BASS_NEVER_TRACE"] = "1"
        bkr = run_bass_kernel_spmd(nc, in_maps, list(range(NCORES)))
    _last_bkr = bkr

    w_new = np.empty((B, K, V), np.float32)
    out = np.empty((B, V), np.float32)
    sar = np.zeros(BL, bool)
    for s in aset:
        sar[GI * s:GI * (s + 1)] = True
    for c in range(NCORES):
        idx = c * BL + perms[c]
        wc = bkr.results[c]["w_out"].reshape(K, BL, V).transpose(1, 0, 2)
        w_new[idx[sar]] = wc[sar]
        w_new[idx[~sar]] = w_assoc[idx[~sar]]
        out[idx] = bkr.results[c]["outp"]
    return out, w_new
